# revision 1
# baseline (speedup 1.0000x reference)
"""CausalWanSelfAttention Trainium2 kernel — single SPMD launch on 8 NeuronCores.

Sharding: column-parallel QKV by heads. Each core owns 2 heads: one exclusive
"F" head plus one boundary "H" head shared with a sibling core; the H head's
output-projection weight is pre-scaled by 0.5 (and its RMSNorm sum-of-squares
contribution weighted 0.5) so summing the 8 partial outputs / statistics is
exact. RMSNorm statistics are combined with one tiny cross-core AllReduce
(2x3712 floats). The block-sparse mask decomposes into 4 dense attention
groups (no masking inside a group), so softmax runs without max-subtraction
(scores are O(1) after RMSNorm; |s| <= sqrt(128)). Scores are computed in
[kv, q] layout; softmax denominators via a ones-matmul; per-query
normalization is fused into the PSUM->SBUF copy. Head dims are permuted
(even dims then odd dims) host-side so RoPE needs no strided ops. State
tokens attend only to themselves (softmax==1 -> o=v): handled on host from a
tiny exported v_state. Heavy matmuls run as float32r (full-rate fp32 mode).
"""
import sys
import numpy as np

sys.path.insert(0, "/opt/trn_rl_repo")

# ---- problem constants (hardcoded; kernel.py must be self-contained) ----
FS = 512
NIB = 3
NAPB = 32
L = 3683
LP = 3712           # 29 * 128
D = 1536
NH = 12
HD = 128
EPS = 1e-6
IB0 = FS                  # 512  image blocks start
A0 = FS + NIB * 2 * FS    # 3584 actions start
S0 = A0 + NIB * NAPB      # 3680 states start
NKT = D // 128            # 12 contraction tiles
NLT = LP // 128           # 29 L tiles
SCALE = float(1.0 / np.sqrt(HD))

CW0 = 384  # projection L-chunk width
CW2 = 256  # rope/normalize L-chunk width


def _mk_chunks(w):
    ch = [(i * w, w) for i in range(LP // w)] + [(LP - LP % w, LP % w)]
    return [(c, x) for (c, x) in ch if x > 0]

CHUNKS = _mk_chunks(CW0)
CHUNKS2 = _mk_chunks(CW2)

# core -> (F head, H head); H heads are computed on two cores each
CORE_HEADS = []
for _a in range(4):
    CORE_HEADS.append((3 * _a, 3 * _a + 1))
    CORE_HEADS.append((3 * _a + 2, 3 * _a + 1))


def _groups():
    """Dense attention groups: q ranges, kv 128-tile indices, runt kv info."""
    gs = [dict(q=[(0, 512)], kvt=list(range(4)), runt=None)]
    for b in range(NIB):
        be = IB0 + (b + 1) * 2 * FS
        kv0 = max(IB0, be - 4 * FS)
        if kv0 == IB0:
            tiles = list(range(be // 128))
        else:
            tiles = list(range(4)) + list(range(kv0 // 128, be // 128))
        q = [(IB0 + b * 2 * FS, 512), (IB0 + b * 2 * FS + 512, 512),
             (A0 + b * NAPB, NAPB)]
        gs.append(dict(q=q, kvt=tiles, runt=b))
    return gs

GROUPS = _groups()

_PROGRAM_CACHE = {}


def _build_program():
    import concourse.bacc as bacc
    import concourse.tile as tile
    from concourse import mybir

    F32 = mybir.dt.float32
    F32R = mybir.dt.float32r
    AF = mybir.ActivationFunctionType

    nc = bacc.Bacc("TRN2", target_bir_lowering=False, debug=False, num_devices=8)

    xT = nc.dram_tensor("xT", [D, LP], F32, kind="ExternalInput")
    wq = nc.dram_tensor("wq", [D, 256], F32, kind="ExternalInput")
    wk = nc.dram_tensor("wk", [D, 256], F32, kind="ExternalInput")
    wv = nc.dram_tensor("wv", [D, 256], F32, kind="ExternalInput")
    wo = nc.dram_tensor("wo", [128, 3072], F32, kind="ExternalInput")
    bqk = nc.dram_tensor("bqk", [128, 4], F32, kind="ExternalInput")
    bv128 = nc.dram_tensor("bv128", [128, 256], F32, kind="ExternalInput")
    cos_d = nc.dram_tensor("cos128", [128, LP], F32, kind="ExternalInput")
    sin_d = nc.dram_tensor("sin128", [128, LP], F32, kind="ExternalInput")
    ones2_d = nc.dram_tensor("ones2", [128, 2], F32, kind="ExternalInput")

    outp = nc.dram_tensor("outp", [D, S0], F32, kind="ExternalOutput")
    vst = nc.dram_tensor("vst", [3, 256], F32, kind="ExternalOutput")

    with tile.TileContext(nc) as tc:
        with tc.tile_pool(name="persist", bufs=1) as P, \
             tc.tile_pool(name="xin", bufs=2) as XP, \
             tc.tile_pool(name="tmp", bufs=2) as T, \
             tc.tile_pool(name="pt", bufs=3) as PT, \
             tc.tile_pool(name="osb", bufs=2) as OSB, \
             tc.tile_pool(name="ps", bufs=2, space="PSUM") as PSY, \
             tc.tile_pool(name="dram", bufs=1, space="DRAM") as DR:

            # ---------- phase-1-resident SBUF ----------
            wq_sb = P.tile([128, NKT, 256], F32R, tag="wq")
            wk_sb = P.tile([128, NKT, 256], F32R, tag="wk")
            wv_sb = P.tile([128, NKT, 256], F32R, tag="wv")
            bqk_sb = P.tile([128, 4], F32, tag="bqk")
            bv_sb = P.tile([128, 256], F32, tag="bv")
            ones2 = P.tile([128, 2], F32R, tag="ones2")
            # whole-kernel-resident
            y_q = [P.tile([128, LP], F32R, tag=f"yq{u}", name=f"yq{u}") for u in range(2)]
            y_k = [P.tile([128, LP], F32R, tag=f"yk{u}", name=f"yk{u}") for u in range(2)]
            v_sb = P.tile([128, NLT, 256], F32R, tag="vsb")

            def ldw(dst, src):
                nc.sync.dma_start(
                    dst[:],
                    src.rearrange("(kt p) c -> p kt c", p=128).bitcast(F32R))

            ldw(wq_sb, wq)
            ldw(wk_sb, wk)
            ldw(wv_sb, wv)
            nc.sync.dma_start(bqk_sb[:], bqk.ap())
            nc.sync.dma_start(bv_sb[:], bv128.ap())
            nc.sync.dma_start(ones2[:], ones2_d.ap().bitcast(F32R))

            # ---------- phase 1: projections + ssq partials ----------
            cin = DR.tile([1, 2 * LP], F32)
            cout = DR.tile([1, 2 * LP], F32)
            xTr = xT.rearrange("(kt p) l -> p kt l", p=128)
            for (c0, cw) in CHUNKS:
                xc = XP.tile([128, NKT, CW0], F32R, tag="xc")
                nc.sync.dma_start(xc[:, :, 0:cw], xTr[:, :, c0:c0 + cw].bitcast(F32R))
                for ti, (w_sb, ys) in enumerate([(wq_sb, y_q), (wk_sb, y_k)]):
                    ssq_ps = PSY.tile([1, 512], F32, tag="ssqps")
                    for u in range(2):
                        yp = PSY.tile([128, 512], F32, tag="yp")
                        for kt in range(NKT):
                            nc.tensor.matmul(
                                yp[:, 0:cw], w_sb[:, kt, u * 128:(u + 1) * 128],
                                xc[:, kt, 0:cw],
                                start=(kt == 0), stop=(kt == NKT - 1))
                        nc.vector.tensor_scalar_add(
                            ys[u][:, c0:c0 + cw], yp[:, 0:cw],
                            bqk_sb[:, 2 * ti + u:2 * ti + u + 1])
                        y2 = T.tile([128, CW0], F32R, tag="y2")
                        nc.scalar.activation(y2[:, 0:cw],
                                             ys[u][:, c0:c0 + cw].bitcast(F32),
                                             AF.Square)
                        nc.tensor.matmul(ssq_ps[:, 0:cw], ones2[:, u:u + 1],
                                         y2[:, 0:cw], start=(u == 0), stop=(u == 1),
                                         skip_group_check=True)
                    ssq_st = T.tile([1, CW0], F32, tag="ssqst")
                    nc.vector.tensor_copy(ssq_st[:, 0:cw], ssq_ps[:, 0:cw])
                    nc.sync.dma_start(cin[0:1, ti * LP + c0:ti * LP + c0 + cw], ssq_st[:, 0:cw])
                for lt in range(c0 // 128, (c0 + cw) // 128):
                    vp = PSY.tile([128, 512], F32, tag="vp", name="vp")[:, 0:256]
                    loff = lt * 128 - c0
                    for kt in range(NKT):
                        nc.tensor.matmul(vp[:], xc[:, kt, loff:loff + 128],
                                         wv_sb[:, kt, :],
                                         start=(kt == 0), stop=(kt == NKT - 1))
                    nc.vector.tensor_add(v_sb[:, lt, :], vp[:], bv_sb[:])

            nc.sync.dma_start(vst.ap(), v_sb[96:99, 28, :].bitcast(F32))

            # ---------- collective: AllReduce the ssq partials ----------
            nc.gpsimd.collective_compute(
                "AllReduce", mybir.AluOpType.add,
                replica_groups=[list(range(8))],
                ins=[cin.opt()], outs=[cout.opt()])
            eps_t = P.tile([1, 1], F32, tag="epst")
            nc.vector.memset(eps_t[:], float(EPS))

            # cos/sin (pair-duplicated across both halves) reuse weight slots
            cos_sb = P.tile([128, LP], F32, tag="wk", name="cos_sb")
            nc.sync.dma_start(cos_sb[:], cos_d.ap())
            sin_sb = P.tile([128, LP], F32, tag="wv", name="sin_sb")
            nc.sync.dma_start(sin_sb[:], sin_d.ap())

            # ---------- phase 2: normalize + rope (in place on y) ----------
            for (c0, cw) in CHUNKS2:
                for ti, ys in enumerate([y_q, y_k]):
                    s1 = T.tile([1, CW2], F32, tag="s1")
                    nc.sync.dma_start(s1[:, 0:cw],
                                      cout[0:1, ti * LP + c0:ti * LP + c0 + cw])
                    nc.scalar.activation(s1[:, 0:cw], s1[:, 0:cw], AF.Sqrt,
                                         bias=eps_t[:, 0:1], scale=float(1.0 / D))
                    nc.vector.reciprocal(s1[:, 0:cw], s1[:, 0:cw])
                    fb = T.tile([128, CW2], F32, tag="fb")
                    nc.gpsimd.partition_broadcast(fb[:, 0:cw], s1[:, 0:cw])
                    for u in range(2):
                        y = ys[u]
                        nc.vector.tensor_mul(y[:, c0:c0 + cw],
                                             y[:, c0:c0 + cw].bitcast(F32),
                                             fb[:, 0:cw])
                        ta = T.tile([128, CW2], F32, tag="ropea")
                        tb = T.tile([128, CW2], F32, tag="ropeb")
                        tbs = T.tile([128, CW2], F32, tag="ropec")
                        yv = y[:, c0:c0 + cw].bitcast(F32)
                        nc.vector.tensor_mul(ta[:, 0:cw], yv, cos_sb[:, c0:c0 + cw])
                        nc.vector.tensor_mul(tb[:, 0:cw], yv, sin_sb[:, c0:c0 + cw])
                        nc.sync.dma_start(tbs[0:64, 0:cw], tb[64:128, 0:cw])
                        nc.sync.dma_start(tbs[64:128, 0:cw], tb[0:64, 0:cw])
                        nc.vector.tensor_sub(y[0:64, c0:c0 + cw],
                                             ta[0:64, 0:cw], tbs[0:64, 0:cw])
                        nc.vector.tensor_add(y[64:128, c0:c0 + cw],
                                             ta[64:128, 0:cw], tbs[64:128, 0:cw])

            # Wo reuses the wq weight slot
            wo_sb = P.tile([128, 3072], F32R, tag="wq", name="wo_sb")
            nc.sync.dma_start(wo_sb[:], wo.ap().bitcast(F32R))

            # ---------- phase 3: attention + partial o-projection ----------
            outr = outp.rearrange("(mt p) l -> p mt l", p=128)
            for g in GROUPS:
                runts = []
                if g["runt"] is not None:
                    b = g["runt"]
                    a_lo = A0 + b * NAPB
                    s_row = S0 + b
                    for u in range(2):
                        kr = T.tile([128, 33], F32R, tag=f"kr{u}")
                        nc.vector.tensor_copy(kr[:, 0:32],
                                              y_k[u][:, a_lo:a_lo + 32].bitcast(F32))
                        nc.vector.tensor_copy(kr[:, 32:33],
                                              y_k[u][:, s_row:s_row + 1].bitcast(F32))
                        vr = T.tile([33, 256], F32R, tag=f"vr{u}")
                        # partition-shifting copies must go through DMA
                        nc.sync.dma_start(
                            vr[0:32, :], v_sb[32 * b:32 * b + 32, 28, :])
                        nc.sync.dma_start(
                            vr[32:33, :], v_sb[96 + b:97 + b, 28, :])
                        runts.append((kr, vr))

                kvts = g["kvt"] + ([None] if g["runt"] is not None else [])
                for (q0, qw) in g["q"]:
                    o_sb = []
                    for u in range(2):
                        oT_ps = PSY.tile([128, 512], F32, tag="vp", name="oT_ps")
                        sm_ps = PSY.tile([1, 512], F32, tag="ssqps", name="sm_ps")
                        for i, t in enumerate(kvts):
                            if t is None:
                                klhs = runts[u][0][:, :]
                                vlhs = runts[u][1][:, u * 128:(u + 1) * 128]
                                kvn = 33
                            else:
                                klhs = y_k[u][:, t * 128:(t + 1) * 128]
                                vlhs = v_sb[:, t, u * 128:(u + 1) * 128]
                                kvn = 128
                            s_ps = PSY.tile([128, 512], F32, tag="yp", name="s_ps")
                            nc.tensor.matmul(s_ps[0:kvn, 0:qw], klhs,
                                             y_q[u][:, q0:q0 + qw],
                                             start=True, stop=True)
                            pT = PT.tile([128, 512], F32R, tag="pT")
                            nc.scalar.activation(pT[0:kvn, 0:qw],
                                                 s_ps[0:kvn, 0:qw], AF.Exp,
                                                 scale=SCALE)
                            nc.tensor.matmul(oT_ps[:, 0:qw], vlhs, pT[0:kvn, 0:qw],
                                             start=(i == 0), stop=(i == len(kvts) - 1),
                                             skip_group_check=True)
                            nc.tensor.matmul(sm_ps[:, 0:qw], ones2[0:kvn, 0:1],
                                             pT[0:kvn, 0:qw],
                                             start=(i == 0), stop=(i == len(kvts) - 1),
                                             skip_group_check=True)
                        sm_sb = T.tile([1, 512], F32, tag="smsb")
                        nc.vector.reciprocal(sm_sb[:, 0:qw], sm_ps[:, 0:qw])
                        rb = T.tile([128, 512], F32, tag="rb")
                        nc.gpsimd.partition_broadcast(rb[:, 0:qw], sm_sb[:, 0:qw])
                        ot = OSB.tile([128, 512], F32R, tag="ot")
                        nc.vector.tensor_mul(ot[:, 0:qw], oT_ps[:, 0:qw], rb[:, 0:qw])
                        o_sb.append(ot)
                    for m in range(NKT):
                        op_ps = PSY.tile([128, 512], F32, tag="op", name="op_ps")
                        for u in range(2):
                            nc.tensor.matmul(
                                op_ps[:, 0:qw],
                                wo_sb[:, u * D + m * 128:u * D + (m + 1) * 128],
                                o_sb[u][:, 0:qw],
                                start=(u == 0), stop=(u == 1))
                        op_sb = OSB.tile([128, 512], F32, tag="opsb")
                        nc.vector.tensor_copy(op_sb[:, 0:qw], op_ps[:, 0:qw])
                        nc.sync.dma_start(outr[:, m, q0:q0 + qw], op_sb[:, 0:qw])

    nc.finalize()
    return nc


def _prep_inputs(x, freqs, freqs_action, freqs_state, Wq, bq, Wk, bk, Wv, bv,
                 Wo, bo, gq, gk):
    """Host-side input prep -> per-core in_maps. gq/gk are ones (per spec)."""
    x = np.ascontiguousarray(np.asarray(x, np.float32)[0])
    xT = np.zeros((D, LP), np.float32)
    xT[:, :L] = x.T
    f = np.concatenate([np.asarray(freqs), np.asarray(freqs_action),
                        np.asarray(freqs_state)], 0).astype(np.float32)
    f = f.reshape(L, HD // 2, 2)
    cos128 = np.zeros((128, LP), np.float32)
    sin128 = np.zeros((128, LP), np.float32)
    cos128[0:64, :L] = f[..., 0].T
    cos128[64:128, :L] = f[..., 0].T
    sin128[0:64, :L] = f[..., 1].T
    sin128[64:128, :L] = f[..., 1].T
    perm = np.concatenate([np.arange(0, HD, 2), np.arange(1, HD, 2)])
    ones2 = np.ones((128, 2), np.float32)
    ones2[:, 1] = 0.5

    Wq = np.asarray(Wq, np.float32); Wk = np.asarray(Wk, np.float32)
    Wv = np.asarray(Wv, np.float32); Wo = np.asarray(Wo, np.float32)
    bq = np.asarray(bq, np.float32); bk = np.asarray(bk, np.float32)
    bv = np.asarray(bv, np.float32)

    in_maps = []
    for c in range(8):
        F, H = CORE_HEADS[c]
        pf = F * HD + perm
        ph = H * HD + perm
        vcols = np.r_[F * HD:(F + 1) * HD, H * HD:(H + 1) * HD]
        in_maps.append({
            "xT": xT,
            "wq": np.ascontiguousarray(np.concatenate([Wq[:, pf], Wq[:, ph]], 1)),
            "wk": np.ascontiguousarray(np.concatenate([Wk[:, pf], Wk[:, ph]], 1)),
            "wv": np.ascontiguousarray(Wv[:, vcols]),
            "wo": np.ascontiguousarray(np.concatenate(
                [Wo[F * HD:(F + 1) * HD, :], 0.5 * Wo[H * HD:(H + 1) * HD, :]],
                1).astype(np.float32)),
            "bqk": np.ascontiguousarray(
                np.stack([bq[pf], bq[ph], bk[pf], bk[ph]], 1).astype(np.float32)),
            "bv128": np.ascontiguousarray(
                np.broadcast_to(bv[vcols][None, :], (128, 256))).copy(),
            "cos128": cos128, "sin128": sin128, "ones2": ones2,
        })
    return in_maps


def kernel(**inputs) -> np.ndarray:
    from concourse.bass_utils import run_bass_kernel_spmd

    if "nc" not in _PROGRAM_CACHE:
        _PROGRAM_CACHE["nc"] = _build_program()
    nc = _PROGRAM_CACHE["nc"]

    in_maps = _prep_inputs(**inputs)
    res = run_bass_kernel_spmd(nc, in_maps, core_ids=list(range(8)))

    Wo = np.asarray(inputs["Wo"], np.float32)
    bo = np.asarray(inputs["bo"], np.float32)
    out = np.zeros((L, D), np.float32)
    acc = np.zeros((D, S0), np.float32)
    for c in range(8):
        acc += res.results[c]["outp"]
    out[:S0] = acc.T
    v_state = np.zeros((3, D), np.float32)
    have = set()
    for c in range(8):
        F, H = CORE_HEADS[c]
        vs = res.results[c]["vst"]
        if F not in have:
            v_state[:, F * HD:(F + 1) * HD] = vs[:, :HD]
            have.add(F)
        if H not in have:
            v_state[:, H * HD:(H + 1) * HD] = vs[:, HD:]
            have.add(H)
    out[S0:S0 + NIB] = v_state @ Wo
    out += bo[None, :]
    return out[None].astype(np.float32)



# revision 4
# speedup vs baseline: 6.9069x; 6.9069x over previous
"""CausalWanSelfAttention Trainium2 kernel — single SPMD launch on 8 NeuronCores.

Sharding: column-parallel QKV by heads. Each core owns 2 heads: one exclusive
"F" head plus one boundary "H" head shared with a sibling core; the H head's
output-projection weight is pre-scaled by 0.5 (and its RMSNorm sum-of-squares
contribution weighted 0.5) so summing the 8 partial outputs / statistics is
exact. RMSNorm statistics are combined with one tiny cross-core AllReduce
(2x3712 floats). The block-sparse mask decomposes into 4 dense attention
groups (no masking inside a group), so softmax runs without max-subtraction
(scores are O(1) after RMSNorm; |s| <= sqrt(128)). Scores are computed in
[kv, q] layout; softmax denominators via a ones-matmul; per-query
normalization is fused into the PSUM->SBUF copy. Head dims are permuted
(even dims then odd dims) host-side so RoPE needs no strided ops. State
tokens attend only to themselves (softmax==1 -> o=v): handled on host from a
tiny exported v_state.

Host<->device traffic is minimized (the launch is transfer-bound over the
axon tunnel): x/cos/sin are shipped fp16 SHARDED over tokens (1/8 per core,
packed into one tensor) and AllGathered on-device; weights ship fp16
per-core; the partial o-projection outputs are combined on-device with a
fp16 ReduceScatter so each core returns only a 1/8 row-slice. Projection
matmuls run fp16 (fp32 PSUM accumulation); attention runs float32r.
"""
import sys
import numpy as np

sys.path.insert(0, "/opt/trn_rl_repo")

# ---- problem constants (hardcoded; kernel.py must be self-contained) ----
FS = 512
NIB = 3
NAPB = 32
L = 3683
LP = 3712           # 29 * 128
D = 1536
NH = 12
HD = 128
EPS = 1e-6
IB0 = FS                  # 512  image blocks start
A0 = FS + NIB * 2 * FS    # 3584 actions start
S0 = A0 + NIB * NAPB      # 3680 states start
NKT = D // 128            # 12 contraction tiles
NLT = LP // 128           # 29 L tiles
SCALE = float(1.0 / np.sqrt(HD))

SW = 512                  # per-core token shard width (8*512 = 4096, padded)
NS = 8
RW = [SW] * 7 + [LP - 7 * SW]   # real token width per shard (last: 128)
GROWS = D + 64 + 64       # gathered rows per shard: x(1536) + cos64 + sin64
ORD = D // 8              # 192 output rows per core after ReduceScatter

CW2 = 256  # rope/normalize L-chunk width


def _mk_chunks(w):
    ch = [(i * w, w) for i in range(LP // w)] + [(LP - LP % w, LP % w)]
    return [(c, x) for (c, x) in ch if x > 0]

CHUNKS2 = _mk_chunks(CW2)

# core -> (F head, H head); H heads are computed on two cores each
CORE_HEADS = []
for _a in range(4):
    CORE_HEADS.append((3 * _a, 3 * _a + 1))
    CORE_HEADS.append((3 * _a + 2, 3 * _a + 1))


def _groups():
    """Dense attention groups: q ranges, kv 128-tile indices, runt kv info."""
    gs = [dict(q=[(0, 512)], kvt=list(range(4)), runt=None)]
    for b in range(NIB):
        be = IB0 + (b + 1) * 2 * FS
        kv0 = max(IB0, be - 4 * FS)
        if kv0 == IB0:
            tiles = list(range(be // 128))
        else:
            tiles = list(range(4)) + list(range(kv0 // 128, be // 128))
        q = [(IB0 + b * 2 * FS, 512), (IB0 + b * 2 * FS + 512, 512),
             (A0 + b * NAPB, NAPB)]
        gs.append(dict(q=q, kvt=tiles, runt=b))
    return gs

GROUPS = _groups()

_PROGRAM_CACHE = {}


def _build_program():
    import concourse.bacc as bacc
    import concourse.tile as tile
    from concourse import mybir

    F16 = mybir.dt.float16
    F32 = mybir.dt.float32
    F32R = mybir.dt.float32r
    AF = mybir.ActivationFunctionType

    nc = bacc.Bacc("TRN2", target_bir_lowering=False, debug=False, num_devices=8)

    gin = nc.dram_tensor("gin", [GROWS, SW], F16, kind="ExternalInput")
    wq = nc.dram_tensor("wq", [D, 256], F16, kind="ExternalInput")
    wk = nc.dram_tensor("wk", [D, 256], F16, kind="ExternalInput")
    wv = nc.dram_tensor("wv", [D, 256], F16, kind="ExternalInput")
    wo = nc.dram_tensor("wo", [128, 3072], F16, kind="ExternalInput")
    bqk = nc.dram_tensor("bqk", [128, 4], F32, kind="ExternalInput")
    bv1 = nc.dram_tensor("bv1", [1, 256], F32, kind="ExternalInput")
    ones2_d = nc.dram_tensor("ones2", [128, 2], F32, kind="ExternalInput")

    outp = nc.dram_tensor("outp", [ORD, LP], F16, kind="ExternalOutput")
    vst = nc.dram_tensor("vst", [3, 256], F32, kind="ExternalOutput")

    with tile.TileContext(nc) as tc:
        with tc.tile_pool(name="persist", bufs=1) as P, \
             tc.tile_pool(name="xin", bufs=2) as XP, \
             tc.tile_pool(name="tmp", bufs=2) as T, \
             tc.tile_pool(name="pt", bufs=3) as PT, \
             tc.tile_pool(name="osb", bufs=2) as OSB, \
             tc.tile_pool(name="ps", bufs=2, space="PSUM") as PSY, \
             tc.tile_pool(name="dram", bufs=1, space="DRAM") as DR:

            # ---------- phase-1-resident SBUF ----------
            wq_sb = P.tile([128, NKT, 256], F16, tag="wq")
            wk_sb = P.tile([128, NKT, 256], F16, tag="wk")
            wv_sb = P.tile([128, NKT, 256], F16, tag="wv")
            bqk_sb = P.tile([128, 4], F32, tag="bqk")
            bv_sb = P.tile([128, 256], F32, tag="bv")
            ones2 = P.tile([128, 2], F32R, tag="ones2")
            # whole-kernel-resident
            y_q = [P.tile([128, LP], F32R, tag=f"yq{u}", name=f"yq{u}") for u in range(2)]
            y_k = [P.tile([128, LP], F32R, tag=f"yk{u}", name=f"yk{u}") for u in range(2)]
            v_sb = P.tile([128, NLT, 256], F32R, tag="vsb")
            cos_sb = P.tile([128, LP], F32, tag="cosslot", name="cos_sb")
            sin_sb = P.tile([128, LP], F32, tag="sinslot", name="sin_sb")

            def ldw(dst, src):
                nc.sync.dma_start(
                    dst[:], src.rearrange("(kt p) c -> p kt c", p=128))

            ldw(wq_sb, wq)
            ldw(wk_sb, wk)
            ldw(wv_sb, wv)
            nc.sync.dma_start(bqk_sb[:], bqk.ap())
            bv1_sb = P.tile([1, 256], F32, tag="bv1")
            nc.sync.dma_start(bv1_sb[:], bv1.ap())
            nc.gpsimd.partition_broadcast(bv_sb[:], bv1_sb[:])
            nc.sync.dma_start(ones2[:], ones2_d.ap().bitcast(F32R))

            # ---------- collective: AllGather x/cos/sin shards ----------
            # (collectives cannot touch IO tensors directly -> stage via
            # internal DRAM)
            gstage = DR.tile([GROWS, SW], F16)
            nc.sync.dma_start(gstage[:], gin.ap())
            ging = DR.tile([NS * GROWS, SW], F16, addr_space="Shared")
            nc.gpsimd.collective_compute(
                "AllGather", mybir.AluOpType.bypass,
                replica_groups=[list(range(8))],
                ins=[gstage.opt()], outs=[ging])

            # cos/sin: fp16 gathered [64, rw] per shard -> duplicated halves,
            # converted to fp32
            for s in range(NS):
                c0, rw = s * SW, RW[s]
                for j, dst in enumerate((cos_sb, sin_sb)):
                    src = ging[s * GROWS + D + 64 * j:s * GROWS + D + 64 * (j + 1), :]
                    cst = T.tile([128, SW], F16, tag="cst")
                    nc.sync.dma_start(cst[0:64, 0:rw], src[:, 0:rw])
                    nc.sync.dma_start(cst[64:128, 0:rw], src[:, 0:rw])
                    nc.vector.tensor_copy(dst[:, c0:c0 + rw], cst[:, 0:rw])

            # ---------- phase 1: projections + ssq partials ----------
            cin = DR.tile([1, 2 * LP], F32)
            cout = DR.tile([1, 2 * LP], F32)
            for s in range(NS):
                c0, rw = s * SW, RW[s]
                xpart = ging[s * GROWS:s * GROWS + D, :].rearrange(
                    "(kt p) l -> p kt l", p=128)
                xc = XP.tile([128, NKT, SW], F16, tag="xc")
                nc.sync.dma_start(xc[:, :, 0:rw], xpart[:, :, 0:rw])
                for ti, (w_sb, ys) in enumerate([(wq_sb, y_q), (wk_sb, y_k)]):
                    ssq_ps = PSY.tile([1, 512], F32, tag="ssqps")
                    for u in range(2):
                        yp = PSY.tile([128, 512], F32, tag="yp")
                        for kt in range(NKT):
                            nc.tensor.matmul(
                                yp[:, 0:rw], w_sb[:, kt, u * 128:(u + 1) * 128],
                                xc[:, kt, 0:rw],
                                start=(kt == 0), stop=(kt == NKT - 1))
                        nc.vector.tensor_scalar_add(
                            ys[u][:, c0:c0 + rw], yp[:, 0:rw],
                            bqk_sb[:, 2 * ti + u:2 * ti + u + 1])
                        y2 = T.tile([128, SW], F32R, tag="y2")
                        nc.scalar.activation(y2[:, 0:rw],
                                             ys[u][:, c0:c0 + rw].bitcast(F32),
                                             AF.Square)
                        nc.tensor.matmul(ssq_ps[:, 0:rw], ones2[:, u:u + 1],
                                         y2[:, 0:rw], start=(u == 0), stop=(u == 1),
                                         skip_group_check=True)
                    ssq_st = T.tile([1, SW], F32, tag="ssqst")
                    nc.vector.tensor_copy(ssq_st[:, 0:rw], ssq_ps[:, 0:rw])
                    nc.sync.dma_start(cin[0:1, ti * LP + c0:ti * LP + c0 + rw], ssq_st[:, 0:rw])
                for lt in range(c0 // 128, (c0 + rw) // 128):
                    vp = PSY.tile([128, 512], F32, tag="vp", name="vp")[:, 0:256]
                    loff = lt * 128 - c0
                    for kt in range(NKT):
                        nc.tensor.matmul(vp[:], xc[:, kt, loff:loff + 128],
                                         wv_sb[:, kt, :],
                                         start=(kt == 0), stop=(kt == NKT - 1))
                    nc.vector.tensor_add(v_sb[:, lt, :], vp[:], bv_sb[:])

            nc.sync.dma_start(vst.ap(), v_sb[96:99, 28, :].bitcast(F32))

            # ---------- collective: AllReduce the ssq partials ----------
            nc.gpsimd.collective_compute(
                "AllReduce", mybir.AluOpType.add,
                replica_groups=[list(range(8))],
                ins=[cin.opt()], outs=[cout.opt()])
            eps_t = P.tile([1, 1], F32, tag="epst")
            nc.vector.memset(eps_t[:], float(EPS))

            # ---------- phase 2: normalize + rope (in place on y) ----------
            for (c0, cw) in CHUNKS2:
                for ti, ys in enumerate([y_q, y_k]):
                    s1 = T.tile([1, CW2], F32, tag="s1")
                    nc.sync.dma_start(s1[:, 0:cw],
                                      cout[0:1, ti * LP + c0:ti * LP + c0 + cw])
                    nc.scalar.activation(s1[:, 0:cw], s1[:, 0:cw], AF.Sqrt,
                                         bias=eps_t[:, 0:1], scale=float(1.0 / D))
                    nc.vector.reciprocal(s1[:, 0:cw], s1[:, 0:cw])
                    fb = T.tile([128, CW2], F32, tag="fb")
                    nc.gpsimd.partition_broadcast(fb[:, 0:cw], s1[:, 0:cw])
                    for u in range(2):
                        y = ys[u]
                        nc.vector.tensor_mul(y[:, c0:c0 + cw],
                                             y[:, c0:c0 + cw].bitcast(F32),
                                             fb[:, 0:cw])
                        ta = T.tile([128, CW2], F32, tag="ropea")
                        tb = T.tile([128, CW2], F32, tag="ropeb")
                        tbs = T.tile([128, CW2], F32, tag="ropec")
                        yv = y[:, c0:c0 + cw].bitcast(F32)
                        nc.vector.tensor_mul(ta[:, 0:cw], yv, cos_sb[:, c0:c0 + cw])
                        nc.vector.tensor_mul(tb[:, 0:cw], yv, sin_sb[:, c0:c0 + cw])
                        nc.sync.dma_start(tbs[0:64, 0:cw], tb[64:128, 0:cw])
                        nc.sync.dma_start(tbs[64:128, 0:cw], tb[0:64, 0:cw])
                        nc.vector.tensor_sub(y[0:64, c0:c0 + cw],
                                             ta[0:64, 0:cw], tbs[0:64, 0:cw])
                        nc.vector.tensor_add(y[64:128, c0:c0 + cw],
                                             ta[64:128, 0:cw], tbs[64:128, 0:cw])

            # Wo arrives fp16, upconverted into the cos_sb slot (free after
            # phase 2)
            wo16_sb = XP.tile([128, 3072], F16, tag="xc", name="wo16_sb")
            nc.sync.dma_start(wo16_sb[:], wo.ap())
            wo_sb = P.tile([128, 3072], F32R, tag="cosslot", name="wo_sb")
            nc.vector.tensor_copy(wo_sb[:], wo16_sb[:])

            # ---------- phase 3: attention + partial o-projection ----------
            outp_loc = DR.tile([D, LP], F16)
            outr = outp_loc.rearrange("(mt p) l -> p mt l", p=128)
            for g in GROUPS:
                runts = []
                if g["runt"] is not None:
                    b = g["runt"]
                    a_lo = A0 + b * NAPB
                    s_row = S0 + b
                    for u in range(2):
                        kr = T.tile([128, 33], F32R, tag=f"kr{u}")
                        nc.vector.tensor_copy(kr[:, 0:32],
                                              y_k[u][:, a_lo:a_lo + 32].bitcast(F32))
                        nc.vector.tensor_copy(kr[:, 32:33],
                                              y_k[u][:, s_row:s_row + 1].bitcast(F32))
                        vr = T.tile([33, 256], F32R, tag=f"vr{u}")
                        # partition-shifting copies must go through DMA
                        nc.sync.dma_start(
                            vr[0:32, :], v_sb[32 * b:32 * b + 32, 28, :])
                        nc.sync.dma_start(
                            vr[32:33, :], v_sb[96 + b:97 + b, 28, :])
                        runts.append((kr, vr))

                kvts = g["kvt"] + ([None] if g["runt"] is not None else [])
                for (q0, qw) in g["q"]:
                    o_sb = []
                    for u in range(2):
                        oT_ps = PSY.tile([128, 512], F32, tag="vp", name="oT_ps")
                        sm_ps = PSY.tile([1, 512], F32, tag="ssqps", name="sm_ps")
                        for i, t in enumerate(kvts):
                            if t is None:
                                klhs = runts[u][0][:, :]
                                vlhs = runts[u][1][:, u * 128:(u + 1) * 128]
                                kvn = 33
                            else:
                                klhs = y_k[u][:, t * 128:(t + 1) * 128]
                                vlhs = v_sb[:, t, u * 128:(u + 1) * 128]
                                kvn = 128
                            s_ps = PSY.tile([128, 512], F32, tag="yp", name="s_ps")
                            nc.tensor.matmul(s_ps[0:kvn, 0:qw], klhs,
                                             y_q[u][:, q0:q0 + qw],
                                             start=True, stop=True)
                            pT = PT.tile([128, 512], F32R, tag="pT")
                            nc.scalar.activation(pT[0:kvn, 0:qw],
                                                 s_ps[0:kvn, 0:qw], AF.Exp,
                                                 scale=SCALE)
                            nc.tensor.matmul(oT_ps[:, 0:qw], vlhs, pT[0:kvn, 0:qw],
                                             start=(i == 0), stop=(i == len(kvts) - 1),
                                             skip_group_check=True)
                            nc.tensor.matmul(sm_ps[:, 0:qw], ones2[0:kvn, 0:1],
                                             pT[0:kvn, 0:qw],
                                             start=(i == 0), stop=(i == len(kvts) - 1),
                                             skip_group_check=True)
                        sm_sb = T.tile([1, 512], F32, tag="smsb")
                        nc.vector.reciprocal(sm_sb[:, 0:qw], sm_ps[:, 0:qw])
                        rb = T.tile([128, 512], F32, tag="rb")
                        nc.gpsimd.partition_broadcast(rb[:, 0:qw], sm_sb[:, 0:qw])
                        ot = OSB.tile([128, 512], F32R, tag="ot")
                        nc.vector.tensor_mul(ot[:, 0:qw], oT_ps[:, 0:qw], rb[:, 0:qw])
                        o_sb.append(ot)
                    for m in range(NKT):
                        op_ps = PSY.tile([128, 512], F32, tag="op", name="op_ps")
                        for u in range(2):
                            nc.tensor.matmul(
                                op_ps[:, 0:qw],
                                wo_sb[:, u * D + m * 128:u * D + (m + 1) * 128],
                                o_sb[u][:, 0:qw],
                                start=(u == 0), stop=(u == 1))
                        op_sb = OSB.tile([128, 512], F16, tag="opsb")
                        nc.vector.tensor_copy(op_sb[:, 0:qw], op_ps[:, 0:qw])
                        nc.sync.dma_start(outr[:, m, q0:q0 + qw], op_sb[:, 0:qw])

            # zero the 3680:3712 pad columns so the ReduceScatter output is
            # garbage-free, then combine partial outputs on-device
            z16 = P.tile([128, 32], F16, tag="z16")
            nc.vector.memset(z16[:], 0.0)
            for m in range(NKT):
                nc.sync.dma_start(outr[:, m, S0:LP], z16[:])
            rs_out = DR.tile([ORD, LP], F16)
            nc.gpsimd.collective_compute(
                "ReduceScatter", mybir.AluOpType.add,
                replica_groups=[list(range(8))],
                ins=[outp_loc.opt()], outs=[rs_out.opt()])
            nc.sync.dma_start(outp.ap(), rs_out[:])

    nc.finalize()
    return nc


def _prep_inputs(x, freqs, freqs_action, freqs_state, Wq, bq, Wk, bk, Wv, bv,
                 Wo, bo, gq, gk):
    """Host-side input prep -> per-core in_maps. gq/gk are ones (per spec)."""
    x = np.asarray(x, np.float32)[0]
    xT16 = np.zeros((D, NS * SW), np.float16)
    xT16[:, :L] = x.T
    f = np.concatenate([np.asarray(freqs), np.asarray(freqs_action),
                        np.asarray(freqs_state)], 0).astype(np.float32)
    f = f.reshape(L, HD // 2, 2)
    cos64 = np.zeros((64, NS * SW), np.float16)
    sin64 = np.zeros((64, NS * SW), np.float16)
    cos64[:, :L] = f[..., 0].T
    sin64[:, :L] = f[..., 1].T
    perm = np.concatenate([np.arange(0, HD, 2), np.arange(1, HD, 2)])
    ones2 = np.ones((128, 2), np.float32)
    ones2[:, 1] = 0.5

    Wq = np.asarray(Wq, np.float32); Wk = np.asarray(Wk, np.float32)
    Wv = np.asarray(Wv, np.float32); Wo = np.asarray(Wo, np.float32)
    bq = np.asarray(bq, np.float32); bk = np.asarray(bk, np.float32)
    bv = np.asarray(bv, np.float32)

    in_maps = []
    for c in range(8):
        F, H = CORE_HEADS[c]
        pf = F * HD + perm
        ph = H * HD + perm
        vcols = np.r_[F * HD:(F + 1) * HD, H * HD:(H + 1) * HD]
        sl = slice(c * SW, (c + 1) * SW)
        in_maps.append({
            "gin": np.ascontiguousarray(np.concatenate(
                [xT16[:, sl], cos64[:, sl], sin64[:, sl]], 0)),
            "wq": np.concatenate([Wq[:, pf], Wq[:, ph]], 1).astype(np.float16),
            "wk": np.concatenate([Wk[:, pf], Wk[:, ph]], 1).astype(np.float16),
            "wv": Wv[:, vcols].astype(np.float16),
            "wo": np.concatenate(
                [Wo[F * HD:(F + 1) * HD, :], 0.5 * Wo[H * HD:(H + 1) * HD, :]],
                1).astype(np.float16),
            "bqk": np.ascontiguousarray(
                np.stack([bq[pf], bq[ph], bk[pf], bk[ph]], 1).astype(np.float32)),
            "bv1": np.ascontiguousarray(bv[vcols][None, :].astype(np.float32)),
            "ones2": ones2,
        })
    return in_maps


def kernel(**inputs) -> np.ndarray:
    from concourse.bass_utils import run_bass_kernel_spmd

    if "nc" not in _PROGRAM_CACHE:
        _PROGRAM_CACHE["nc"] = _build_program()
    nc = _PROGRAM_CACHE["nc"]

    in_maps = _prep_inputs(**inputs)
    res = run_bass_kernel_spmd(nc, in_maps, core_ids=list(range(8)))

    Wo = np.asarray(inputs["Wo"], np.float32)
    bo = np.asarray(inputs["bo"], np.float32)
    acc = np.concatenate([res.results[c]["outp"] for c in range(8)],
                         0).astype(np.float32)
    out = np.zeros((L, D), np.float32)
    out[:S0] = acc[:, :S0].T
    v_state = np.zeros((3, D), np.float32)
    have = set()
    for c in range(8):
        F, H = CORE_HEADS[c]
        vs = res.results[c]["vst"]
        if F not in have:
            v_state[:, F * HD:(F + 1) * HD] = vs[:, :HD]
            have.add(F)
        if H not in have:
            v_state[:, H * HD:(H + 1) * HD] = vs[:, HD:]
            have.add(H)
    out[S0:S0 + NIB] = v_state @ Wo
    out += bo[None, :]
    return out[None].astype(np.float32)


# revision 6
# speedup vs baseline: 6.9653x; 1.0085x over previous
"""CausalWanSelfAttention Trainium2 kernel — single SPMD launch on 8 NeuronCores.

Sharding: column-parallel QKV by heads. Each core owns 2 heads: one exclusive
"F" head plus one boundary "H" head shared with a sibling core; the H head's
output-projection weight is pre-scaled by 0.5 (and its RMSNorm sum-of-squares
contribution weighted 0.5) so summing the 8 partial outputs / statistics is
exact. RMSNorm statistics are combined with one tiny cross-core AllReduce
(2x3712 floats). The block-sparse mask decomposes into 4 dense attention
groups (no masking inside a group), so softmax runs without max-subtraction
(scores are O(1) after RMSNorm; |s| <= sqrt(128)). Scores are computed in
[kv, q] layout; softmax denominators via a ones-matmul; per-query
normalization is fused into the PSUM->SBUF copy. Head dims are permuted
(even dims then odd dims) host-side so RoPE needs no strided ops. State
tokens attend only to themselves (softmax==1 -> o=v): their three output
columns are produced on-chip from v_state^T = Wv^T x_state^T.

Host<->device traffic is minimized (the launch is transfer-bound over the
axon tunnel): x/cos/sin/biases ship fp16 SHARDED over tokens (1/8 per core,
packed into one tensor) and are AllGathered on-device; all four weight
matrices ship fp16 packed in one per-core tensor; the partial o-projection
outputs are combined on-device with an fp16 ReduceScatter so each core
returns only a 1/8 row-slice. Projection matmuls run fp16 (fp32 PSUM
accumulation); attention runs float32r.
"""
import sys
import numpy as np

sys.path.insert(0, "/opt/trn_rl_repo")

# ---- problem constants (hardcoded; kernel.py must be self-contained) ----
FS = 512
NIB = 3
NAPB = 32
L = 3683
LP = 3712           # 29 * 128
D = 1536
NH = 12
HD = 128
EPS = 1e-6
IB0 = FS                  # 512  image blocks start
A0 = FS + NIB * 2 * FS    # 3584 actions start
S0 = A0 + NIB * NAPB      # 3680 states start
NKT = D // 128            # 12 contraction tiles
NLT = LP // 128           # 29 L tiles
SCALE = float(1.0 / np.sqrt(HD))

SW = 512                  # per-core token shard width (8*512 = 4096, padded)
NS = 8
RW = [SW] * 7 + [LP - 7 * SW]   # real token width per shard (last: 128)
GROWS = D + 64 + 64 + 2   # shard rows: x(1536) + cos64 + sin64 + bv + bqk
ORD = D // 8              # 192 output rows per core after ReduceScatter

CW2 = 256  # rope/normalize L-chunk width


def _mk_chunks(w):
    ch = [(i * w, w) for i in range(LP // w)] + [(LP - LP % w, LP % w)]
    return [(c, x) for (c, x) in ch if x > 0]

CHUNKS2 = _mk_chunks(CW2)

# core -> (F head, H head); H heads are computed on two cores each
CORE_HEADS = []
for _a in range(4):
    CORE_HEADS.append((3 * _a, 3 * _a + 1))
    CORE_HEADS.append((3 * _a + 2, 3 * _a + 1))


def _groups():
    """Dense attention groups: q ranges, kv 128-tile indices, runt kv info."""
    gs = [dict(q=[(0, 512)], kvt=list(range(4)), runt=None)]
    for b in range(NIB):
        be = IB0 + (b + 1) * 2 * FS
        kv0 = max(IB0, be - 4 * FS)
        if kv0 == IB0:
            tiles = list(range(be // 128))
        else:
            tiles = list(range(4)) + list(range(kv0 // 128, be // 128))
        q = [(IB0 + b * 2 * FS, 512), (IB0 + b * 2 * FS + 512, 512),
             (A0 + b * NAPB, NAPB)]
        gs.append(dict(q=q, kvt=tiles, runt=b))
    return gs

GROUPS = _groups()

_PROGRAM_CACHE = {}


def _build_program():
    import concourse.bacc as bacc
    import concourse.tile as tile
    from concourse import mybir

    F16 = mybir.dt.float16
    F32 = mybir.dt.float32
    F32R = mybir.dt.float32r
    AF = mybir.ActivationFunctionType

    nc = bacc.Bacc("TRN2", target_bir_lowering=False, debug=False, num_devices=8)

    gin = nc.dram_tensor("gin", [GROWS, SW], F16, kind="ExternalInput")
    wall = nc.dram_tensor("wall", [4 * D, 256], F16, kind="ExternalInput")

    outp = nc.dram_tensor("outp", [ORD, LP], F16, kind="ExternalOutput")

    with tile.TileContext(nc) as tc:
        with tc.tile_pool(name="persist", bufs=1) as P, \
             tc.tile_pool(name="xin", bufs=2) as XP, \
             tc.tile_pool(name="tmp", bufs=2) as T, \
             tc.tile_pool(name="pt", bufs=3) as PT, \
             tc.tile_pool(name="osb", bufs=2) as OSB, \
             tc.tile_pool(name="ps", bufs=2, space="PSUM") as PSY, \
             tc.tile_pool(name="dram", bufs=1, space="DRAM") as DR:

            # ---------- phase-1-resident SBUF ----------
            wq_sb = P.tile([128, NKT, 256], F16, tag="wq")
            wk_sb = P.tile([128, NKT, 256], F16, tag="wk")
            wv_sb = P.tile([128, NKT, 256], F16, tag="wv")
            bqk_sb = P.tile([128, 4], F32, tag="bqk")
            bv_sb = P.tile([128, 256], F32, tag="bv")
            bvT = P.tile([128, 2], F32, tag="bvT")
            ones2 = P.tile([128, 2], F32, tag="ones2")
            # whole-kernel-resident
            y_q = [P.tile([128, LP], F32R, tag=f"yq{u}", name=f"yq{u}") for u in range(2)]
            y_k = [P.tile([128, LP], F32R, tag=f"yk{u}", name=f"yk{u}") for u in range(2)]
            v_sb = P.tile([128, NLT, 256], F32R, tag="vsb")
            cos_sb = P.tile([128, LP], F32, tag="cosslot", name="cos_sb")
            sin_sb = P.tile([128, LP], F32, tag="sinslot", name="sin_sb")

            for t, w_sb in enumerate((wq_sb, wk_sb, wv_sb)):
                nc.sync.dma_start(
                    w_sb[:], wall.ap()[t * D:(t + 1) * D, :].rearrange(
                        "(kt p) c -> p kt c", p=128))
            # biases arrive fp16 inside this core's own (pre-gather) shard
            bqk16 = T.tile([128, 4], F16, tag="bqk16")
            nc.sync.dma_start(
                bqk16[:], gin.ap()[D + 129, :].rearrange("(i p) -> p i", p=128))
            nc.vector.tensor_copy(bqk_sb[:], bqk16[:])
            bv16 = T.tile([1, 256], F16, tag="bv16")
            nc.sync.dma_start(bv16[:], gin.ap()[D + 128:D + 129, 0:256])
            bv1f = T.tile([1, 256], F32, tag="bv1f")
            nc.vector.tensor_copy(bv1f[:], bv16[:])
            nc.gpsimd.partition_broadcast(bv_sb[:], bv1f[:])
            bvT16 = T.tile([128, 2], F16, tag="bvT16")
            nc.sync.dma_start(
                bvT16[:], gin.ap()[D + 128, 0:256].rearrange("(u p) -> p u", p=128))
            nc.vector.tensor_copy(bvT[:], bvT16[:])
            nc.vector.memset(ones2[:, 0:1], 1.0)
            nc.vector.memset(ones2[:, 1:2], 0.5)

            def ones_r(sl):
                return ones2[sl].bitcast(F32R)

            # ---------- collective: AllGather x/cos/sin shards ----------
            # (collectives cannot touch IO tensors directly -> stage via
            # internal DRAM)
            gstage = DR.tile([GROWS, SW], F16)
            nc.sync.dma_start(gstage[:], gin.ap())
            ging = DR.tile([NS * GROWS, SW], F16, addr_space="Shared")
            nc.gpsimd.collective_compute(
                "AllGather", mybir.AluOpType.bypass,
                replica_groups=[list(range(8))],
                ins=[gstage.opt()], outs=[ging])

            # cos/sin: fp16 gathered [64, rw] per shard -> duplicated halves,
            # converted to fp32
            for s in range(NS):
                c0, rw = s * SW, RW[s]
                for j, dst in enumerate((cos_sb, sin_sb)):
                    src = ging[s * GROWS + D + 64 * j:s * GROWS + D + 64 * (j + 1), :]
                    cst = T.tile([128, SW], F16, tag="cst")
                    nc.sync.dma_start(cst[0:64, 0:rw], src[:, 0:rw])
                    nc.sync.dma_start(cst[64:128, 0:rw], src[:, 0:rw])
                    nc.vector.tensor_copy(dst[:, c0:c0 + rw], cst[:, 0:rw])

            # ---------- phase 1: projections + ssq partials ----------
            cin = DR.tile([1, 2 * LP], F32)
            cout = DR.tile([1, 2 * LP], F32)
            for s in range(NS):
                c0, rw = s * SW, RW[s]
                xpart = ging[s * GROWS:s * GROWS + D, :].rearrange(
                    "(kt p) l -> p kt l", p=128)
                xc = XP.tile([128, NKT, SW], F16, tag="xc")
                nc.sync.dma_start(xc[:, :, 0:rw], xpart[:, :, 0:rw])
                for ti, (w_sb, ys) in enumerate([(wq_sb, y_q), (wk_sb, y_k)]):
                    ssq_ps = PSY.tile([1, 512], F32, tag="ssqps")
                    for u in range(2):
                        yp = PSY.tile([128, 512], F32, tag="yp")
                        for kt in range(NKT):
                            nc.tensor.matmul(
                                yp[:, 0:rw], w_sb[:, kt, u * 128:(u + 1) * 128],
                                xc[:, kt, 0:rw],
                                start=(kt == 0), stop=(kt == NKT - 1))
                        nc.vector.tensor_scalar_add(
                            ys[u][:, c0:c0 + rw], yp[:, 0:rw],
                            bqk_sb[:, 2 * ti + u:2 * ti + u + 1])
                        y2 = T.tile([128, SW], F32R, tag="y2")
                        nc.scalar.activation(y2[:, 0:rw],
                                             ys[u][:, c0:c0 + rw].bitcast(F32),
                                             AF.Square)
                        nc.tensor.matmul(ssq_ps[:, 0:rw], ones_r((slice(None), slice(u, u + 1))),
                                         y2[:, 0:rw], start=(u == 0), stop=(u == 1),
                                         skip_group_check=True)
                    ssq_st = T.tile([1, SW], F32, tag="ssqst")
                    nc.vector.tensor_copy(ssq_st[:, 0:rw], ssq_ps[:, 0:rw])
                    nc.sync.dma_start(cin[0:1, ti * LP + c0:ti * LP + c0 + rw], ssq_st[:, 0:rw])
                for lt in range(c0 // 128, (c0 + rw) // 128):
                    vp = PSY.tile([128, 512], F32, tag="vp", name="vp")[:, 0:256]
                    loff = lt * 128 - c0
                    for kt in range(NKT):
                        nc.tensor.matmul(vp[:], xc[:, kt, loff:loff + 128],
                                         wv_sb[:, kt, :],
                                         start=(kt == 0), stop=(kt == NKT - 1))
                    nc.vector.tensor_add(v_sb[:, lt, :], vp[:], bv_sb[:])

            # state tokens (3680:3683, in shard 7 cols 96:99): o = v, computed
            # transposed as v^T = Wv^T x^T so it feeds the o-projection directly
            xst = T.tile([128, NKT, 4], F16, tag="xst")
            nc.sync.dma_start(
                xst[:], ging[7 * GROWS:7 * GROWS + D, :].rearrange(
                    "(kt p) l -> p kt l", p=128)[:, :, 96:100])
            o_state = [P.tile([128, 4], F32R, tag=f"ost{u}", name=f"ost{u}")
                       for u in range(2)]
            for u in range(2):
                vs_ps = PSY.tile([128, 512], F32, tag="yp", name="vs_ps")
                for kt in range(NKT):
                    nc.tensor.matmul(vs_ps[:, 0:4],
                                     wv_sb[:, kt, u * 128:(u + 1) * 128],
                                     xst[:, kt, :],
                                     start=(kt == 0), stop=(kt == NKT - 1))
                nc.vector.tensor_scalar_add(o_state[u][:], vs_ps[:, 0:4],
                                            bvT[:, u:u + 1])

            # ---------- collective: AllReduce the ssq partials ----------
            nc.gpsimd.collective_compute(
                "AllReduce", mybir.AluOpType.add,
                replica_groups=[list(range(8))],
                ins=[cin.opt()], outs=[cout.opt()])
            eps_t = P.tile([1, 1], F32, tag="epst")
            nc.vector.memset(eps_t[:], float(EPS))

            # ---------- phase 2: normalize + rope (in place on y) ----------
            for (c0, cw) in CHUNKS2:
                for ti, ys in enumerate([y_q, y_k]):
                    s1 = T.tile([1, CW2], F32, tag="s1")
                    nc.sync.dma_start(s1[:, 0:cw],
                                      cout[0:1, ti * LP + c0:ti * LP + c0 + cw])
                    nc.scalar.activation(s1[:, 0:cw], s1[:, 0:cw], AF.Sqrt,
                                         bias=eps_t[:, 0:1], scale=float(1.0 / D))
                    nc.vector.reciprocal(s1[:, 0:cw], s1[:, 0:cw])
                    fb = T.tile([128, CW2], F32, tag="fb")
                    nc.gpsimd.partition_broadcast(fb[:, 0:cw], s1[:, 0:cw])
                    for u in range(2):
                        y = ys[u]
                        nc.vector.tensor_mul(y[:, c0:c0 + cw],
                                             y[:, c0:c0 + cw].bitcast(F32),
                                             fb[:, 0:cw])
                        ta = T.tile([128, CW2], F32, tag="ropea")
                        tb = T.tile([128, CW2], F32, tag="ropeb")
                        tbs = T.tile([128, CW2], F32, tag="ropec")
                        yv = y[:, c0:c0 + cw].bitcast(F32)
                        nc.vector.tensor_mul(ta[:, 0:cw], yv, cos_sb[:, c0:c0 + cw])
                        nc.vector.tensor_mul(tb[:, 0:cw], yv, sin_sb[:, c0:c0 + cw])
                        nc.sync.dma_start(tbs[0:64, 0:cw], tb[64:128, 0:cw])
                        nc.sync.dma_start(tbs[64:128, 0:cw], tb[0:64, 0:cw])
                        nc.vector.tensor_sub(y[0:64, c0:c0 + cw],
                                             ta[0:64, 0:cw], tbs[0:64, 0:cw])
                        nc.vector.tensor_add(y[64:128, c0:c0 + cw],
                                             ta[64:128, 0:cw], tbs[64:128, 0:cw])

            # Wo arrives fp16 packed as [128,12,256]; upconvert into the
            # cos_sb slot (free after phase 2)
            wo16_sb = XP.tile([128, NKT, 256], F16, tag="xc", name="wo16_sb")
            nc.sync.dma_start(
                wo16_sb[:], wall.ap()[3 * D:4 * D, :].rearrange(
                    "(p j) c -> p j c", p=128))
            wo_sb = P.tile([128, NKT, 256], F32R, tag="cosslot", name="wo_sb")
            nc.vector.tensor_copy(wo_sb[:], wo16_sb[:])

            def wo_slice(u, m):
                # flat columns [u*D + m*128, +128) in (j, c) coords
                j = u * 6 + m // 2
                c = (m % 2) * 128
                return wo_sb[:, j, c:c + 128]

            # ---------- phase 3: attention + partial o-projection ----------
            outp_loc = DR.tile([D, LP], F16)
            outr = outp_loc.rearrange("(mt p) l -> p mt l", p=128)
            for g in GROUPS:
                runts = []
                if g["runt"] is not None:
                    b = g["runt"]
                    a_lo = A0 + b * NAPB
                    s_row = S0 + b
                    for u in range(2):
                        kr = T.tile([128, 33], F32R, tag=f"kr{u}")
                        nc.vector.tensor_copy(kr[:, 0:32],
                                              y_k[u][:, a_lo:a_lo + 32].bitcast(F32))
                        nc.vector.tensor_copy(kr[:, 32:33],
                                              y_k[u][:, s_row:s_row + 1].bitcast(F32))
                        vr = T.tile([33, 256], F32R, tag=f"vr{u}")
                        # partition-shifting copies must go through DMA
                        nc.sync.dma_start(
                            vr[0:32, :], v_sb[32 * b:32 * b + 32, 28, :])
                        nc.sync.dma_start(
                            vr[32:33, :], v_sb[96 + b:97 + b, 28, :])
                        runts.append((kr, vr))

                kvts = g["kvt"] + ([None] if g["runt"] is not None else [])
                for (q0, qw) in g["q"]:
                    o_sb = []
                    for u in range(2):
                        oT_ps = PSY.tile([128, 512], F32, tag="vp", name="oT_ps")
                        sm_ps = PSY.tile([1, 512], F32, tag="ssqps", name="sm_ps")
                        for i, t in enumerate(kvts):
                            if t is None:
                                klhs = runts[u][0][:, :]
                                vlhs = runts[u][1][:, u * 128:(u + 1) * 128]
                                kvn = 33
                            else:
                                klhs = y_k[u][:, t * 128:(t + 1) * 128]
                                vlhs = v_sb[:, t, u * 128:(u + 1) * 128]
                                kvn = 128
                            s_ps = PSY.tile([128, 512], F32, tag="yp", name="s_ps")
                            nc.tensor.matmul(s_ps[0:kvn, 0:qw], klhs,
                                             y_q[u][:, q0:q0 + qw],
                                             start=True, stop=True)
                            pT = PT.tile([128, 512], F32R, tag="pT")
                            nc.scalar.activation(pT[0:kvn, 0:qw],
                                                 s_ps[0:kvn, 0:qw], AF.Exp,
                                                 scale=SCALE)
                            nc.tensor.matmul(oT_ps[:, 0:qw], vlhs, pT[0:kvn, 0:qw],
                                             start=(i == 0), stop=(i == len(kvts) - 1),
                                             skip_group_check=True)
                            nc.tensor.matmul(sm_ps[:, 0:qw], ones_r((slice(0, kvn), slice(0, 1))),
                                             pT[0:kvn, 0:qw],
                                             start=(i == 0), stop=(i == len(kvts) - 1),
                                             skip_group_check=True)
                        sm_sb = T.tile([1, 512], F32, tag="smsb")
                        nc.vector.reciprocal(sm_sb[:, 0:qw], sm_ps[:, 0:qw])
                        rb = T.tile([128, 512], F32, tag="rb")
                        nc.gpsimd.partition_broadcast(rb[:, 0:qw], sm_sb[:, 0:qw])
                        ot = OSB.tile([128, 512], F32R, tag="ot")
                        nc.vector.tensor_mul(ot[:, 0:qw], oT_ps[:, 0:qw], rb[:, 0:qw])
                        o_sb.append(ot)
                    for m in range(NKT):
                        op_ps = PSY.tile([128, 512], F32, tag="op", name="op_ps")
                        for u in range(2):
                            nc.tensor.matmul(
                                op_ps[:, 0:qw], wo_slice(u, m), o_sb[u][:, 0:qw],
                                start=(u == 0), stop=(u == 1))
                        op_sb = OSB.tile([128, 512], F16, tag="opsb")
                        nc.vector.tensor_copy(op_sb[:, 0:qw], op_ps[:, 0:qw])
                        nc.sync.dma_start(outr[:, m, q0:q0 + qw], op_sb[:, 0:qw])

            # state-token columns (3680:3683, plus discarded pad col
            # 3683: fp32r matmuls need width >= 4): o-projection of v_state^T
            for m in range(NKT):
                op_ps = PSY.tile([128, 512], F32, tag="op", name="op_ps_st")
                for u in range(2):
                    nc.tensor.matmul(op_ps[:, 0:4], wo_slice(u, m),
                                     o_state[u][:], start=(u == 0), stop=(u == 1))
                op_sb = OSB.tile([128, 512], F16, tag="opsb")
                nc.vector.tensor_copy(op_sb[:, 0:4], op_ps[:, 0:4])
                nc.sync.dma_start(outr[:, m, S0:S0 + 4], op_sb[:, 0:4])

            # zero the 3683:3712 pad columns so the ReduceScatter output is
            # garbage-free, then combine partial outputs on-device
            z16 = P.tile([128, 28], F16, tag="z16")
            nc.vector.memset(z16[:], 0.0)
            for m in range(NKT):
                nc.sync.dma_start(outr[:, m, L + 1:LP], z16[:])
            rs_out = DR.tile([ORD, LP], F16)
            nc.gpsimd.collective_compute(
                "ReduceScatter", mybir.AluOpType.add,
                replica_groups=[list(range(8))],
                ins=[outp_loc.opt()], outs=[rs_out.opt()])
            nc.sync.dma_start(outp.ap(), rs_out[:])

    nc.finalize()
    return nc


def _prep_inputs(x, freqs, freqs_action, freqs_state, Wq, bq, Wk, bk, Wv, bv,
                 Wo, bo, gq, gk):
    """Host-side input prep -> per-core in_maps. gq/gk are ones (per spec)."""
    x = np.asarray(x, np.float32)[0]
    xT16 = np.zeros((D, NS * SW), np.float16)
    xT16[:, :L] = x.T
    f = np.concatenate([np.asarray(freqs), np.asarray(freqs_action),
                        np.asarray(freqs_state)], 0).astype(np.float32)
    f = f.reshape(L, HD // 2, 2)
    cs16 = np.zeros((128, NS * SW), np.float16)
    cs16[0:64, :L] = f[..., 0].T
    cs16[64:128, :L] = f[..., 1].T
    perm = np.concatenate([np.arange(0, HD, 2), np.arange(1, HD, 2)])

    Wq = np.asarray(Wq, np.float32); Wk = np.asarray(Wk, np.float32)
    Wv = np.asarray(Wv, np.float32); Wo = np.asarray(Wo, np.float32)
    bq = np.asarray(bq, np.float32); bk = np.asarray(bk, np.float32)
    bv = np.asarray(bv, np.float32)

    in_maps = []
    for c in range(8):
        F, H = CORE_HEADS[c]
        pf = F * HD + perm
        ph = H * HD + perm
        vcols = np.r_[F * HD:(F + 1) * HD, H * HD:(H + 1) * HD]
        sl = slice(c * SW, (c + 1) * SW)
        brows = np.zeros((2, SW), np.float16)
        brows[0, 0:256] = bv[vcols]
        brows[1, :] = np.concatenate([bq[pf], bq[ph], bk[pf], bk[ph]])
        wo16 = np.concatenate(
            [Wo[F * HD:(F + 1) * HD, :], 0.5 * Wo[H * HD:(H + 1) * HD, :]],
            1).astype(np.float16)        # [128, 3072]
        in_maps.append({
            "gin": np.ascontiguousarray(np.concatenate(
                [xT16[:, sl], cs16[:, sl], brows], 0)),
            "wall": np.ascontiguousarray(np.concatenate(
                [np.concatenate([Wq[:, pf], Wq[:, ph]], 1).astype(np.float16),
                 np.concatenate([Wk[:, pf], Wk[:, ph]], 1).astype(np.float16),
                 Wv[:, vcols].astype(np.float16),
                 wo16.reshape(D, 256)], 0)),
        })
    return in_maps


def kernel(**inputs) -> np.ndarray:
    from concourse.bass_utils import run_bass_kernel_spmd

    if "nc" not in _PROGRAM_CACHE:
        _PROGRAM_CACHE["nc"] = _build_program()
    nc = _PROGRAM_CACHE["nc"]

    in_maps = _prep_inputs(**inputs)
    res = run_bass_kernel_spmd(nc, in_maps, core_ids=list(range(8)))

    bo = np.asarray(inputs["bo"], np.float32)
    acc = np.concatenate([res.results[c]["outp"] for c in range(8)],
                         0).astype(np.float32)
    out = acc[:, :L].T + bo[None, :]
    return out[None].astype(np.float32)


# revision 8
# speedup vs baseline: 7.7566x; 1.1136x over previous
"""CausalWanSelfAttention Trainium2 kernel — single SPMD launch on 8 NeuronCores.

Sharding: column-parallel QKV by heads. Each core owns 2 heads: one exclusive
"F" head plus one boundary "H" head shared with a sibling core; the H head's
output-projection weight is pre-scaled by 0.5 (and its RMSNorm sum-of-squares
contribution weighted 0.5) so summing the 8 partial outputs / statistics is
exact. RMSNorm statistics are combined with one tiny cross-core AllReduce
(2x3712 floats). The block-sparse mask decomposes into 4 dense attention
groups (no masking inside a group), so softmax runs without max-subtraction
(scores are O(1) after RMSNorm; |s| <= sqrt(128)). Scores are computed in
[kv, q] layout; softmax denominators via a ones-matmul; per-query
normalization is fused into the PSUM->SBUF copy. Head dims are permuted
(even dims then odd dims) host-side so RoPE needs no strided ops. State
tokens attend only to themselves (softmax==1 -> o=v): their three output
columns are produced on-chip from v_state^T = Wv^T x_state^T.

Host<->device traffic is minimized (the launch is transfer-bound over the
axon tunnel): x/cos/sin/biases ship fp16 SHARDED over tokens (1/8 per core,
packed into one tensor) and are AllGathered on-device; all four weight
matrices ship fp16 packed in one per-core tensor; the partial o-projection
outputs are combined on-device with an fp16 ReduceScatter so each core
returns only a 1/8 row-slice. Projection matmuls run fp16 (fp32 PSUM
accumulation); attention runs float32r.
"""
import sys
import numpy as np

sys.path.insert(0, "/opt/trn_rl_repo")

# ---- problem constants (hardcoded; kernel.py must be self-contained) ----
FS = 512
NIB = 3
NAPB = 32
L = 3683
LP = 3712           # 29 * 128
D = 1536
NH = 12
HD = 128
EPS = 1e-6
IB0 = FS                  # 512  image blocks start
A0 = FS + NIB * 2 * FS    # 3584 actions start
S0 = A0 + NIB * NAPB      # 3680 states start
NKT = D // 128            # 12 contraction tiles
NLT = LP // 128           # 29 L tiles
SCALE = float(1.0 / np.sqrt(HD))

SW = 512                  # per-core token shard width (8*512 = 4096, padded)
NS = 8
RW = [SW] * 7 + [LP - 7 * SW]   # real token width per shard (last: 128)
GROWS = D + 64 + 64 + 2   # shard rows: x(1536) + cos64 + sin64 + bv + bqk
ORD = D // 8              # 192 output rows per core after ReduceScatter

CW2 = 256  # rope/normalize L-chunk width


def _mk_chunks(w):
    ch = [(i * w, w) for i in range(LP // w)] + [(LP - LP % w, LP % w)]
    return [(c, x) for (c, x) in ch if x > 0]

CHUNKS2 = _mk_chunks(CW2)

# core -> (F head, H head); H heads are computed on two cores each
CORE_HEADS = []
for _a in range(4):
    CORE_HEADS.append((3 * _a, 3 * _a + 1))
    CORE_HEADS.append((3 * _a + 2, 3 * _a + 1))


def _groups():
    """Dense attention groups: q ranges, kv 128-tile indices, runt kv info."""
    gs = [dict(q=[(0, 512)], kvt=list(range(4)), runt=None)]
    for b in range(NIB):
        be = IB0 + (b + 1) * 2 * FS
        kv0 = max(IB0, be - 4 * FS)
        if kv0 == IB0:
            tiles = list(range(be // 128))
        else:
            tiles = list(range(4)) + list(range(kv0 // 128, be // 128))
        q = [(IB0 + b * 2 * FS, 512), (IB0 + b * 2 * FS + 512, 512),
             (A0 + b * NAPB, NAPB)]
        gs.append(dict(q=q, kvt=tiles, runt=b))
    return gs

GROUPS = _groups()

_PROGRAM_CACHE = {}


def _build_program():
    import concourse.bacc as bacc
    import concourse.tile as tile
    from concourse import mybir

    F16 = mybir.dt.float16
    F32 = mybir.dt.float32
    F32R = mybir.dt.float32r
    AF = mybir.ActivationFunctionType

    nc = bacc.Bacc("TRN2", target_bir_lowering=False, debug=False, num_devices=8)

    gin = nc.dram_tensor("gin", [GROWS, SW], F16, kind="ExternalInput")
    wall = nc.dram_tensor("wall", [4 * D, 128], F16, kind="ExternalInput")
    whalf = nc.dram_tensor("whalf", [2 * D, 128], F16, kind="ExternalInput")

    outp = nc.dram_tensor("outp", [ORD, LP], F16, kind="ExternalOutput")

    with tile.TileContext(nc) as tc:
        with tc.tile_pool(name="persist", bufs=1) as P, \
             tc.tile_pool(name="xin", bufs=2) as XP, \
             tc.tile_pool(name="tmp", bufs=2) as T, \
             tc.tile_pool(name="pt", bufs=3) as PT, \
             tc.tile_pool(name="osb", bufs=2) as OSB, \
             tc.tile_pool(name="ps", bufs=2, space="PSUM") as PSY, \
             tc.tile_pool(name="dram", bufs=1, space="DRAM") as DR:

            # ---------- phase-1-resident SBUF ----------
            wq_sb = P.tile([128, NKT, 256], F16, tag="wq")
            wk_sb = P.tile([128, NKT, 256], F16, tag="wk")
            wv_sb = P.tile([128, NKT, 256], F16, tag="wv")
            bqk_sb = P.tile([128, 4], F32, tag="bqk")
            bv_sb = P.tile([128, 256], F32, tag="bv")
            bvT = P.tile([128, 2], F32, tag="bvT")
            ones2 = P.tile([128, 2], F32, tag="ones2")
            # whole-kernel-resident
            y_q = [P.tile([128, LP], F32R, tag=f"yq{u}", name=f"yq{u}") for u in range(2)]
            y_k = [P.tile([128, LP], F32R, tag=f"yk{u}", name=f"yk{u}") for u in range(2)]
            v_sb = P.tile([128, NLT, 256], F32R, tag="vsb")
            cos_sb = P.tile([128, LP], F32, tag="cosslot", name="cos_sb")
            sin_sb = P.tile([128, LP], F32, tag="sinslot", name="sin_sb")

            for t, w_sb in enumerate((wq_sb, wk_sb, wv_sb)):
                nc.sync.dma_start(
                    w_sb[:, :, 0:128], wall.ap()[t * D:(t + 1) * D, :].rearrange(
                        "(kt p) c -> p kt c", p=128))
            # biases arrive fp16 inside this core's own (pre-gather) shard
            bqk16 = T.tile([128, 4], F16, tag="bqk16")
            nc.sync.dma_start(
                bqk16[:], gin.ap()[D + 129, :].rearrange("(i p) -> p i", p=128))
            nc.vector.tensor_copy(bqk_sb[:], bqk16[:])
            bv16 = T.tile([1, 256], F16, tag="bv16")
            nc.sync.dma_start(bv16[:], gin.ap()[D + 128:D + 129, 0:256])
            bv1f = T.tile([1, 256], F32, tag="bv1f")
            nc.vector.tensor_copy(bv1f[:], bv16[:])
            nc.gpsimd.partition_broadcast(bv_sb[:], bv1f[:])
            bvT16 = T.tile([128, 2], F16, tag="bvT16")
            nc.sync.dma_start(
                bvT16[:], gin.ap()[D + 128, 0:256].rearrange("(u p) -> p u", p=128))
            nc.vector.tensor_copy(bvT[:], bvT16[:])
            nc.vector.memset(ones2[:, 0:1], 1.0)
            nc.vector.memset(ones2[:, 1:2], 0.5)

            def ones_r(sl):
                return ones2[sl].bitcast(F32R)

            # ---------- collective: AllGather x/cos/sin shards ----------
            # (collectives cannot touch IO tensors directly -> stage via
            # internal DRAM)
            gstage = DR.tile([GROWS, SW], F16)
            nc.sync.dma_start(gstage[:], gin.ap())
            ging = DR.tile([NS * GROWS, SW], F16, addr_space="Shared")
            nc.gpsimd.collective_compute(
                "AllGather", mybir.AluOpType.bypass,
                replica_groups=[list(range(8))],
                ins=[gstage.opt()], outs=[ging])

            # shared H-head weights: each pair member ships half, pair-wise
            # AllGather reconstructs [hq; hk; hv; hwo] identically on both
            whstage = DR.tile([2 * D, 128], F16)
            nc.sync.dma_start(whstage[:], whalf.ap())
            hfull = DR.tile([4 * D, 128], F16)
            nc.gpsimd.collective_compute(
                "AllGather", mybir.AluOpType.bypass,
                replica_groups=[[2 * a, 2 * a + 1] for a in range(4)],
                ins=[whstage.opt()], outs=[hfull])
            for t, w_sb in enumerate((wq_sb, wk_sb, wv_sb)):
                nc.sync.dma_start(
                    w_sb[:, :, 128:256], hfull[t * D:(t + 1) * D, :].rearrange(
                        "(kt p) c -> p kt c", p=128))

            # cos/sin: fp16 gathered [64, rw] per shard -> duplicated halves,
            # converted to fp32
            for s in range(NS):
                c0, rw = s * SW, RW[s]
                for j, dst in enumerate((cos_sb, sin_sb)):
                    src = ging[s * GROWS + D + 64 * j:s * GROWS + D + 64 * (j + 1), :]
                    cst = T.tile([128, SW], F16, tag="cst")
                    nc.sync.dma_start(cst[0:64, 0:rw], src[:, 0:rw])
                    nc.sync.dma_start(cst[64:128, 0:rw], src[:, 0:rw])
                    nc.vector.tensor_copy(dst[:, c0:c0 + rw], cst[:, 0:rw])

            # ---------- phase 1: projections + ssq partials ----------
            cin = DR.tile([1, 2 * LP], F32)
            cout = DR.tile([1, 2 * LP], F32)
            for s in range(NS):
                c0, rw = s * SW, RW[s]
                xpart = ging[s * GROWS:s * GROWS + D, :].rearrange(
                    "(kt p) l -> p kt l", p=128)
                xc = XP.tile([128, NKT, SW], F16, tag="xc")
                nc.sync.dma_start(xc[:, :, 0:rw], xpart[:, :, 0:rw])
                for ti, (w_sb, ys) in enumerate([(wq_sb, y_q), (wk_sb, y_k)]):
                    ssq_ps = PSY.tile([1, 512], F32, tag="ssqps")
                    for u in range(2):
                        yp = PSY.tile([128, 512], F32, tag="yp")
                        for kt in range(NKT):
                            nc.tensor.matmul(
                                yp[:, 0:rw], w_sb[:, kt, u * 128:(u + 1) * 128],
                                xc[:, kt, 0:rw],
                                start=(kt == 0), stop=(kt == NKT - 1))
                        nc.vector.tensor_scalar_add(
                            ys[u][:, c0:c0 + rw], yp[:, 0:rw],
                            bqk_sb[:, 2 * ti + u:2 * ti + u + 1])
                        y2 = T.tile([128, SW], F32R, tag="y2")
                        nc.scalar.activation(y2[:, 0:rw],
                                             ys[u][:, c0:c0 + rw].bitcast(F32),
                                             AF.Square)
                        nc.tensor.matmul(ssq_ps[:, 0:rw], ones_r((slice(None), slice(u, u + 1))),
                                         y2[:, 0:rw], start=(u == 0), stop=(u == 1),
                                         skip_group_check=True)
                    ssq_st = T.tile([1, SW], F32, tag="ssqst")
                    nc.vector.tensor_copy(ssq_st[:, 0:rw], ssq_ps[:, 0:rw])
                    nc.sync.dma_start(cin[0:1, ti * LP + c0:ti * LP + c0 + rw], ssq_st[:, 0:rw])
                for lt in range(c0 // 128, (c0 + rw) // 128):
                    vp = PSY.tile([128, 512], F32, tag="vp", name="vp")[:, 0:256]
                    loff = lt * 128 - c0
                    for kt in range(NKT):
                        nc.tensor.matmul(vp[:], xc[:, kt, loff:loff + 128],
                                         wv_sb[:, kt, :],
                                         start=(kt == 0), stop=(kt == NKT - 1))
                    nc.vector.tensor_add(v_sb[:, lt, :], vp[:], bv_sb[:])

            # state tokens (3680:3683, in shard 7 cols 96:99): o = v, computed
            # transposed as v^T = Wv^T x^T so it feeds the o-projection directly
            xst = T.tile([128, NKT, 4], F16, tag="xst")
            nc.sync.dma_start(
                xst[:], ging[7 * GROWS:7 * GROWS + D, :].rearrange(
                    "(kt p) l -> p kt l", p=128)[:, :, 96:100])
            o_state = [P.tile([128, 4], F32R, tag=f"ost{u}", name=f"ost{u}")
                       for u in range(2)]
            for u in range(2):
                vs_ps = PSY.tile([128, 512], F32, tag="yp", name="vs_ps")
                for kt in range(NKT):
                    nc.tensor.matmul(vs_ps[:, 0:4],
                                     wv_sb[:, kt, u * 128:(u + 1) * 128],
                                     xst[:, kt, :],
                                     start=(kt == 0), stop=(kt == NKT - 1))
                nc.vector.tensor_scalar_add(o_state[u][:], vs_ps[:, 0:4],
                                            bvT[:, u:u + 1])

            # ---------- collective: AllReduce the ssq partials ----------
            nc.gpsimd.collective_compute(
                "AllReduce", mybir.AluOpType.add,
                replica_groups=[list(range(8))],
                ins=[cin.opt()], outs=[cout.opt()])
            eps_t = P.tile([1, 1], F32, tag="epst")
            nc.vector.memset(eps_t[:], float(EPS))

            # ---------- phase 2: normalize + rope (in place on y) ----------
            for (c0, cw) in CHUNKS2:
                for ti, ys in enumerate([y_q, y_k]):
                    s1 = T.tile([1, CW2], F32, tag="s1")
                    nc.sync.dma_start(s1[:, 0:cw],
                                      cout[0:1, ti * LP + c0:ti * LP + c0 + cw])
                    nc.scalar.activation(s1[:, 0:cw], s1[:, 0:cw], AF.Sqrt,
                                         bias=eps_t[:, 0:1], scale=float(1.0 / D))
                    nc.vector.reciprocal(s1[:, 0:cw], s1[:, 0:cw])
                    fb = T.tile([128, CW2], F32, tag="fb")
                    nc.gpsimd.partition_broadcast(fb[:, 0:cw], s1[:, 0:cw])
                    for u in range(2):
                        y = ys[u]
                        nc.vector.tensor_mul(y[:, c0:c0 + cw],
                                             y[:, c0:c0 + cw].bitcast(F32),
                                             fb[:, 0:cw])
                        ta = T.tile([128, CW2], F32, tag="ropea")
                        tb = T.tile([128, CW2], F32, tag="ropeb")
                        tbs = T.tile([128, CW2], F32, tag="ropec")
                        yv = y[:, c0:c0 + cw].bitcast(F32)
                        nc.vector.tensor_mul(ta[:, 0:cw], yv, cos_sb[:, c0:c0 + cw])
                        nc.vector.tensor_mul(tb[:, 0:cw], yv, sin_sb[:, c0:c0 + cw])
                        nc.sync.dma_start(tbs[0:64, 0:cw], tb[64:128, 0:cw])
                        nc.sync.dma_start(tbs[64:128, 0:cw], tb[0:64, 0:cw])
                        nc.vector.tensor_sub(y[0:64, c0:c0 + cw],
                                             ta[0:64, 0:cw], tbs[0:64, 0:cw])
                        nc.vector.tensor_add(y[64:128, c0:c0 + cw],
                                             ta[64:128, 0:cw], tbs[64:128, 0:cw])

            # Wo arrives fp16 packed as [128,12,256]; upconvert into the
            # cos_sb slot (free after phase 2)
            wo16_sb = XP.tile([128, 2 * NKT, 128], F16, tag="xc", name="wo16_sb")
            nc.sync.dma_start(
                wo16_sb[:, 0:NKT, :], wall.ap()[3 * D:4 * D, :].rearrange(
                    "(p j) c -> p j c", p=128))
            nc.sync.dma_start(
                wo16_sb[:, NKT:2 * NKT, :], hfull[3 * D:4 * D, :].rearrange(
                    "(p j) c -> p j c", p=128))
            wo_sb = P.tile([128, 2 * NKT, 128], F32R, tag="cosslot", name="wo_sb")
            nc.vector.tensor_copy(wo_sb[:], wo16_sb[:])

            def wo_slice(u, m):
                return wo_sb[:, u * NKT + m, :]

            # ---------- phase 3: attention + partial o-projection ----------
            outp_loc = DR.tile([D, LP], F16)
            outr = outp_loc.rearrange("(mt p) l -> p mt l", p=128)
            for g in GROUPS:
                runts = []
                if g["runt"] is not None:
                    b = g["runt"]
                    a_lo = A0 + b * NAPB
                    s_row = S0 + b
                    for u in range(2):
                        kr = T.tile([128, 33], F32R, tag=f"kr{u}")
                        nc.vector.tensor_copy(kr[:, 0:32],
                                              y_k[u][:, a_lo:a_lo + 32].bitcast(F32))
                        nc.vector.tensor_copy(kr[:, 32:33],
                                              y_k[u][:, s_row:s_row + 1].bitcast(F32))
                        vr = T.tile([33, 256], F32R, tag=f"vr{u}")
                        # partition-shifting copies must go through DMA
                        nc.sync.dma_start(
                            vr[0:32, :], v_sb[32 * b:32 * b + 32, 28, :])
                        nc.sync.dma_start(
                            vr[32:33, :], v_sb[96 + b:97 + b, 28, :])
                        runts.append((kr, vr))

                kvts = g["kvt"] + ([None] if g["runt"] is not None else [])
                for (q0, qw) in g["q"]:
                    o_sb = []
                    for u in range(2):
                        oT_ps = PSY.tile([128, 512], F32, tag="vp", name="oT_ps")
                        sm_ps = PSY.tile([1, 512], F32, tag="ssqps", name="sm_ps")
                        for i, t in enumerate(kvts):
                            if t is None:
                                klhs = runts[u][0][:, :]
                                vlhs = runts[u][1][:, u * 128:(u + 1) * 128]
                                kvn = 33
                            else:
                                klhs = y_k[u][:, t * 128:(t + 1) * 128]
                                vlhs = v_sb[:, t, u * 128:(u + 1) * 128]
                                kvn = 128
                            s_ps = PSY.tile([128, 512], F32, tag="yp", name="s_ps")
                            nc.tensor.matmul(s_ps[0:kvn, 0:qw], klhs,
                                             y_q[u][:, q0:q0 + qw],
                                             start=True, stop=True)
                            pT = PT.tile([128, 512], F32R, tag="pT")
                            nc.scalar.activation(pT[0:kvn, 0:qw],
                                                 s_ps[0:kvn, 0:qw], AF.Exp,
                                                 scale=SCALE)
                            nc.tensor.matmul(oT_ps[:, 0:qw], vlhs, pT[0:kvn, 0:qw],
                                             start=(i == 0), stop=(i == len(kvts) - 1),
                                             skip_group_check=True)
                            nc.tensor.matmul(sm_ps[:, 0:qw], ones_r((slice(0, kvn), slice(0, 1))),
                                             pT[0:kvn, 0:qw],
                                             start=(i == 0), stop=(i == len(kvts) - 1),
                                             skip_group_check=True)
                        sm_sb = T.tile([1, 512], F32, tag="smsb")
                        nc.vector.reciprocal(sm_sb[:, 0:qw], sm_ps[:, 0:qw])
                        rb = T.tile([128, 512], F32, tag="rb")
                        nc.gpsimd.partition_broadcast(rb[:, 0:qw], sm_sb[:, 0:qw])
                        ot = OSB.tile([128, 512], F32R, tag="ot")
                        nc.vector.tensor_mul(ot[:, 0:qw], oT_ps[:, 0:qw], rb[:, 0:qw])
                        o_sb.append(ot)
                    for m in range(NKT):
                        op_ps = PSY.tile([128, 512], F32, tag="op", name="op_ps")
                        for u in range(2):
                            nc.tensor.matmul(
                                op_ps[:, 0:qw], wo_slice(u, m), o_sb[u][:, 0:qw],
                                start=(u == 0), stop=(u == 1))
                        op_sb = OSB.tile([128, 512], F16, tag="opsb")
                        nc.vector.tensor_copy(op_sb[:, 0:qw], op_ps[:, 0:qw])
                        nc.sync.dma_start(outr[:, m, q0:q0 + qw], op_sb[:, 0:qw])

            # state-token columns (3680:3683, plus discarded pad col
            # 3683: fp32r matmuls need width >= 4): o-projection of v_state^T
            for m in range(NKT):
                op_ps = PSY.tile([128, 512], F32, tag="op", name="op_ps_st")
                for u in range(2):
                    nc.tensor.matmul(op_ps[:, 0:4], wo_slice(u, m),
                                     o_state[u][:], start=(u == 0), stop=(u == 1))
                op_sb = OSB.tile([128, 512], F16, tag="opsb")
                nc.vector.tensor_copy(op_sb[:, 0:4], op_ps[:, 0:4])
                nc.sync.dma_start(outr[:, m, S0:S0 + 4], op_sb[:, 0:4])

            # zero the 3683:3712 pad columns so the ReduceScatter output is
            # garbage-free, then combine partial outputs on-device
            z16 = P.tile([128, 28], F16, tag="z16")
            nc.vector.memset(z16[:], 0.0)
            for m in range(NKT):
                nc.sync.dma_start(outr[:, m, L + 1:LP], z16[:])
            rs_out = DR.tile([ORD, LP], F16)
            nc.gpsimd.collective_compute(
                "ReduceScatter", mybir.AluOpType.add,
                replica_groups=[list(range(8))],
                ins=[outp_loc.opt()], outs=[rs_out.opt()])
            nc.sync.dma_start(outp.ap(), rs_out[:])

    nc.finalize()
    return nc


def _prep_inputs(x, freqs, freqs_action, freqs_state, Wq, bq, Wk, bk, Wv, bv,
                 Wo, bo, gq, gk):
    """Host-side input prep -> per-core in_maps. gq/gk are ones (per spec)."""
    x = np.asarray(x, np.float32)[0]
    xT16 = np.zeros((D, NS * SW), np.float16)
    xT16[:, :L] = x.T
    f = np.concatenate([np.asarray(freqs), np.asarray(freqs_action),
                        np.asarray(freqs_state)], 0).astype(np.float32)
    f = f.reshape(L, HD // 2, 2)
    cs16 = np.zeros((128, NS * SW), np.float16)
    cs16[0:64, :L] = f[..., 0].T
    cs16[64:128, :L] = f[..., 1].T
    perm = np.concatenate([np.arange(0, HD, 2), np.arange(1, HD, 2)])

    Wq = np.asarray(Wq, np.float32); Wk = np.asarray(Wk, np.float32)
    Wv = np.asarray(Wv, np.float32); Wo = np.asarray(Wo, np.float32)
    bq = np.asarray(bq, np.float32); bk = np.asarray(bk, np.float32)
    bv = np.asarray(bv, np.float32)

    in_maps = []
    for c in range(8):
        F, H = CORE_HEADS[c]
        pf = F * HD + perm
        ph = H * HD + perm
        vcols = np.r_[F * HD:(F + 1) * HD, H * HD:(H + 1) * HD]
        sl = slice(c * SW, (c + 1) * SW)
        brows = np.zeros((2, SW), np.float16)
        brows[0, 0:256] = bv[vcols]
        brows[1, :] = np.concatenate([bq[pf], bq[ph], bk[pf], bk[ph]])
        woF = Wo[F * HD:(F + 1) * HD, :].astype(np.float16).reshape(D, 128)
        hq = Wq[:, ph].astype(np.float16)
        hk = Wk[:, ph].astype(np.float16)
        hv = Wv[:, H * HD:(H + 1) * HD].astype(np.float16)
        hwo = (0.5 * Wo[H * HD:(H + 1) * HD, :]).astype(np.float16).reshape(D, 128)
        whalf = (np.concatenate([hq, hk], 0) if c % 2 == 0
                 else np.concatenate([hv, hwo], 0))
        in_maps.append({
            "gin": np.ascontiguousarray(np.concatenate(
                [xT16[:, sl], cs16[:, sl], brows], 0)),
            "wall": np.ascontiguousarray(np.concatenate(
                [Wq[:, pf].astype(np.float16),
                 Wk[:, pf].astype(np.float16),
                 Wv[:, F * HD:(F + 1) * HD].astype(np.float16),
                 woF], 0)),
            "whalf": np.ascontiguousarray(whalf),
        })
    return in_maps


def kernel(**inputs) -> np.ndarray:
    from concourse.bass_utils import run_bass_kernel_spmd

    if "nc" not in _PROGRAM_CACHE:
        _PROGRAM_CACHE["nc"] = _build_program()
    nc = _PROGRAM_CACHE["nc"]

    in_maps = _prep_inputs(**inputs)
    acc = None
    for attempt in range(3):
        try:
            res = run_bass_kernel_spmd(nc, in_maps, core_ids=list(range(8)))
            a = np.concatenate([res.results[c]["outp"] for c in range(8)],
                               0).astype(np.float32)
        except Exception:
            if attempt == 2:
                raise
            continue
        acc = a
        # transient device flakes can surface as non-finite values; relaunch
        if np.isfinite(a[:, :L]).all():
            break
    assert acc is not None

    bo = np.asarray(inputs["bo"], np.float32)
    out = acc[:, :L].T + bo[None, :]
    return out[None].astype(np.float32)


# revision 9
# speedup vs baseline: 7.9480x; 1.0247x over previous
"""CausalWanSelfAttention Trainium2 kernel — single SPMD launch on 8 NeuronCores.

Sharding: column-parallel QKV by heads. Each core owns 2 heads: one exclusive
"F" head plus one boundary "H" head shared with a sibling core; the H head's
output-projection weight is pre-scaled by 0.5 (and its RMSNorm sum-of-squares
contribution weighted 0.5) so summing the 8 partial outputs / statistics is
exact. RMSNorm statistics are combined with one tiny cross-core AllReduce
(2x3712 floats). The block-sparse mask decomposes into 4 dense attention
groups (no masking inside a group), so softmax runs without max-subtraction
(scores are O(1) after RMSNorm; |s| <= sqrt(128)). Scores are computed in
[kv, q] layout; softmax denominators via a ones-matmul; per-query
normalization is fused into the PSUM->SBUF copy. Head dims are permuted
(even dims then odd dims) host-side so RoPE needs no strided ops. State
tokens attend only to themselves (softmax==1 -> o=v): their three output
columns are produced on-chip from v_state^T = Wv^T x_state^T.

Host<->device traffic is minimized (the launch is transfer-bound over the
axon tunnel): x/cos/sin/biases ship fp16 SHARDED over tokens (1/8 per core,
packed into one tensor) and are AllGathered on-device; all four weight
matrices ship fp16 packed in one per-core tensor; the partial o-projection
outputs are combined on-device with an fp16 ReduceScatter so each core
returns only a 1/8 row-slice. Projection matmuls run fp16 (fp32 PSUM
accumulation); attention runs float32r.
"""
import os
import sys
import numpy as np

sys.path.insert(0, "/opt/trn_rl_repo")
# skip NEFF debug-info emission in the per-launch walrus compile (~70ms/launch)
os.environ.setdefault("CONCOURSE_SCRUB_NEFF_DEBUG_INFO", "1")

# ---- problem constants (hardcoded; kernel.py must be self-contained) ----
FS = 512
NIB = 3
NAPB = 32
L = 3683
LP = 3712           # 29 * 128
D = 1536
NH = 12
HD = 128
EPS = 1e-6
IB0 = FS                  # 512  image blocks start
A0 = FS + NIB * 2 * FS    # 3584 actions start
S0 = A0 + NIB * NAPB      # 3680 states start
NKT = D // 128            # 12 contraction tiles
NLT = LP // 128           # 29 L tiles
SCALE = float(1.0 / np.sqrt(HD))

SW = 512                  # per-core token shard width (8*512 = 4096, padded)
NS = 8
RW = [SW] * 7 + [LP - 7 * SW]   # real token width per shard (last: 128)
GROWS = D + 64 + 64 + 2   # shard rows: x(1536) + cos64 + sin64 + bv + bqk
ORD = D // 8              # 192 output rows per core after ReduceScatter

CW2 = 256  # rope/normalize L-chunk width


def _mk_chunks(w):
    ch = [(i * w, w) for i in range(LP // w)] + [(LP - LP % w, LP % w)]
    return [(c, x) for (c, x) in ch if x > 0]

CHUNKS2 = _mk_chunks(CW2)

# core -> (F head, H head); H heads are computed on two cores each
CORE_HEADS = []
for _a in range(4):
    CORE_HEADS.append((3 * _a, 3 * _a + 1))
    CORE_HEADS.append((3 * _a + 2, 3 * _a + 1))


def _groups():
    """Dense attention groups: q ranges, kv 128-tile indices, runt kv info."""
    gs = [dict(q=[(0, 512)], kvt=list(range(4)), runt=None)]
    for b in range(NIB):
        be = IB0 + (b + 1) * 2 * FS
        kv0 = max(IB0, be - 4 * FS)
        if kv0 == IB0:
            tiles = list(range(be // 128))
        else:
            tiles = list(range(4)) + list(range(kv0 // 128, be // 128))
        q = [(IB0 + b * 2 * FS, 512), (IB0 + b * 2 * FS + 512, 512),
             (A0 + b * NAPB, NAPB)]
        gs.append(dict(q=q, kvt=tiles, runt=b))
    return gs

GROUPS = _groups()

_PROGRAM_CACHE = {}


def _build_program():
    import concourse.bacc as bacc
    import concourse.tile as tile
    from concourse import mybir

    F16 = mybir.dt.float16
    F32 = mybir.dt.float32
    F32R = mybir.dt.float32r
    AF = mybir.ActivationFunctionType

    nc = bacc.Bacc("TRN2", target_bir_lowering=False, debug=False, num_devices=8)

    gin = nc.dram_tensor("gin", [GROWS, SW], F16, kind="ExternalInput")
    wall = nc.dram_tensor("wall", [4 * D, 128], F16, kind="ExternalInput")
    whalf = nc.dram_tensor("whalf", [2 * D, 128], F16, kind="ExternalInput")

    outp = nc.dram_tensor("outp", [ORD, LP], F16, kind="ExternalOutput")

    with tile.TileContext(nc) as tc:
        with tc.tile_pool(name="persist", bufs=1) as P, \
             tc.tile_pool(name="xin", bufs=2) as XP, \
             tc.tile_pool(name="tmp", bufs=2) as T, \
             tc.tile_pool(name="pt", bufs=3) as PT, \
             tc.tile_pool(name="osb", bufs=2) as OSB, \
             tc.tile_pool(name="ps", bufs=2, space="PSUM") as PSY, \
             tc.tile_pool(name="dram", bufs=1, space="DRAM") as DR:

            # ---------- phase-1-resident SBUF ----------
            wq_sb = P.tile([128, NKT, 256], F16, tag="wq")
            wk_sb = P.tile([128, NKT, 256], F16, tag="wk")
            wv_sb = P.tile([128, NKT, 256], F16, tag="wv")
            bqk_sb = P.tile([128, 4], F32, tag="bqk")
            bv_sb = P.tile([128, 256], F32, tag="bv")
            bvT = P.tile([128, 2], F32, tag="bvT")
            ones2 = P.tile([128, 2], F32, tag="ones2")
            # whole-kernel-resident
            y_q = [P.tile([128, LP], F32R, tag=f"yq{u}", name=f"yq{u}") for u in range(2)]
            y_k = [P.tile([128, LP], F32R, tag=f"yk{u}", name=f"yk{u}") for u in range(2)]
            v_sb = P.tile([128, NLT, 256], F32R, tag="vsb")
            cos_sb = P.tile([128, LP], F32, tag="cosslot", name="cos_sb")
            sin_sb = P.tile([128, LP], F32, tag="sinslot", name="sin_sb")

            for t, w_sb in enumerate((wq_sb, wk_sb, wv_sb)):
                nc.sync.dma_start(
                    w_sb[:, :, 0:128], wall.ap()[t * D:(t + 1) * D, :].rearrange(
                        "(kt p) c -> p kt c", p=128))
            # biases arrive fp16 inside this core's own (pre-gather) shard
            bqk16 = T.tile([128, 4], F16, tag="bqk16")
            nc.sync.dma_start(
                bqk16[:], gin.ap()[D + 129, :].rearrange("(i p) -> p i", p=128))
            nc.vector.tensor_copy(bqk_sb[:], bqk16[:])
            bv16 = T.tile([1, 256], F16, tag="bv16")
            nc.sync.dma_start(bv16[:], gin.ap()[D + 128:D + 129, 0:256])
            bv1f = T.tile([1, 256], F32, tag="bv1f")
            nc.vector.tensor_copy(bv1f[:], bv16[:])
            nc.gpsimd.partition_broadcast(bv_sb[:], bv1f[:])
            bvT16 = T.tile([128, 2], F16, tag="bvT16")
            nc.sync.dma_start(
                bvT16[:], gin.ap()[D + 128, 0:256].rearrange("(u p) -> p u", p=128))
            nc.vector.tensor_copy(bvT[:], bvT16[:])
            nc.vector.memset(ones2[:, 0:1], 1.0)
            nc.vector.memset(ones2[:, 1:2], 0.5)

            def ones_r(sl):
                return ones2[sl].bitcast(F32R)

            # ---------- collective: AllGather x/cos/sin shards ----------
            # (collectives cannot touch IO tensors directly -> stage via
            # internal DRAM)
            gstage = DR.tile([GROWS, SW], F16)
            nc.sync.dma_start(gstage[:], gin.ap())
            ging = DR.tile([NS * GROWS, SW], F16, addr_space="Shared")
            nc.gpsimd.collective_compute(
                "AllGather", mybir.AluOpType.bypass,
                replica_groups=[list(range(8))],
                ins=[gstage.opt()], outs=[ging])

            # shared H-head weights: each pair member ships half, pair-wise
            # AllGather reconstructs [hq; hk; hv; hwo] identically on both
            whstage = DR.tile([2 * D, 128], F16)
            nc.sync.dma_start(whstage[:], whalf.ap())
            hfull = DR.tile([4 * D, 128], F16)
            nc.gpsimd.collective_compute(
                "AllGather", mybir.AluOpType.bypass,
                replica_groups=[[2 * a, 2 * a + 1] for a in range(4)],
                ins=[whstage.opt()], outs=[hfull])
            for t, w_sb in enumerate((wq_sb, wk_sb, wv_sb)):
                nc.sync.dma_start(
                    w_sb[:, :, 128:256], hfull[t * D:(t + 1) * D, :].rearrange(
                        "(kt p) c -> p kt c", p=128))

            # cos/sin: fp16 gathered [64, rw] per shard -> duplicated halves,
            # converted to fp32
            for s in range(NS):
                c0, rw = s * SW, RW[s]
                for j, dst in enumerate((cos_sb, sin_sb)):
                    src = ging[s * GROWS + D + 64 * j:s * GROWS + D + 64 * (j + 1), :]
                    cst = T.tile([128, SW], F16, tag="cst")
                    nc.sync.dma_start(cst[0:64, 0:rw], src[:, 0:rw])
                    nc.sync.dma_start(cst[64:128, 0:rw], src[:, 0:rw])
                    nc.vector.tensor_copy(dst[:, c0:c0 + rw], cst[:, 0:rw])

            # ---------- phase 1: projections + ssq partials ----------
            cin = DR.tile([1, 2 * LP], F32)
            cout = DR.tile([1, 2 * LP], F32)
            for s in range(NS):
                c0, rw = s * SW, RW[s]
                xpart = ging[s * GROWS:s * GROWS + D, :].rearrange(
                    "(kt p) l -> p kt l", p=128)
                xc = XP.tile([128, NKT, SW], F16, tag="xc")
                nc.sync.dma_start(xc[:, :, 0:rw], xpart[:, :, 0:rw])
                for ti, (w_sb, ys) in enumerate([(wq_sb, y_q), (wk_sb, y_k)]):
                    ssq_ps = PSY.tile([1, 512], F32, tag="ssqps")
                    for u in range(2):
                        yp = PSY.tile([128, 512], F32, tag="yp")
                        for kt in range(NKT):
                            nc.tensor.matmul(
                                yp[:, 0:rw], w_sb[:, kt, u * 128:(u + 1) * 128],
                                xc[:, kt, 0:rw],
                                start=(kt == 0), stop=(kt == NKT - 1))
                        nc.vector.tensor_scalar_add(
                            ys[u][:, c0:c0 + rw], yp[:, 0:rw],
                            bqk_sb[:, 2 * ti + u:2 * ti + u + 1])
                        y2 = T.tile([128, SW], F32R, tag="y2")
                        nc.scalar.activation(y2[:, 0:rw],
                                             ys[u][:, c0:c0 + rw].bitcast(F32),
                                             AF.Square)
                        nc.tensor.matmul(ssq_ps[:, 0:rw], ones_r((slice(None), slice(u, u + 1))),
                                         y2[:, 0:rw], start=(u == 0), stop=(u == 1),
                                         skip_group_check=True)
                    ssq_st = T.tile([1, SW], F32, tag="ssqst")
                    nc.vector.tensor_copy(ssq_st[:, 0:rw], ssq_ps[:, 0:rw])
                    nc.sync.dma_start(cin[0:1, ti * LP + c0:ti * LP + c0 + rw], ssq_st[:, 0:rw])
                for lt in range(c0 // 128, (c0 + rw) // 128):
                    vp = PSY.tile([128, 512], F32, tag="vp", name="vp")[:, 0:256]
                    loff = lt * 128 - c0
                    for kt in range(NKT):
                        nc.tensor.matmul(vp[:], xc[:, kt, loff:loff + 128],
                                         wv_sb[:, kt, :],
                                         start=(kt == 0), stop=(kt == NKT - 1))
                    nc.vector.tensor_add(v_sb[:, lt, :], vp[:], bv_sb[:])

            # state tokens (3680:3683, in shard 7 cols 96:99): o = v, computed
            # transposed as v^T = Wv^T x^T so it feeds the o-projection directly
            xst = T.tile([128, NKT, 4], F16, tag="xst")
            nc.sync.dma_start(
                xst[:], ging[7 * GROWS:7 * GROWS + D, :].rearrange(
                    "(kt p) l -> p kt l", p=128)[:, :, 96:100])
            o_state = [P.tile([128, 4], F32R, tag=f"ost{u}", name=f"ost{u}")
                       for u in range(2)]
            for u in range(2):
                vs_ps = PSY.tile([128, 512], F32, tag="yp", name="vs_ps")
                for kt in range(NKT):
                    nc.tensor.matmul(vs_ps[:, 0:4],
                                     wv_sb[:, kt, u * 128:(u + 1) * 128],
                                     xst[:, kt, :],
                                     start=(kt == 0), stop=(kt == NKT - 1))
                nc.vector.tensor_scalar_add(o_state[u][:], vs_ps[:, 0:4],
                                            bvT[:, u:u + 1])

            # ---------- collective: AllReduce the ssq partials ----------
            nc.gpsimd.collective_compute(
                "AllReduce", mybir.AluOpType.add,
                replica_groups=[list(range(8))],
                ins=[cin.opt()], outs=[cout.opt()])
            eps_t = P.tile([1, 1], F32, tag="epst")
            nc.vector.memset(eps_t[:], float(EPS))

            # ---------- phase 2: normalize + rope (in place on y) ----------
            for (c0, cw) in CHUNKS2:
                for ti, ys in enumerate([y_q, y_k]):
                    s1 = T.tile([1, CW2], F32, tag="s1")
                    nc.sync.dma_start(s1[:, 0:cw],
                                      cout[0:1, ti * LP + c0:ti * LP + c0 + cw])
                    nc.scalar.activation(s1[:, 0:cw], s1[:, 0:cw], AF.Sqrt,
                                         bias=eps_t[:, 0:1], scale=float(1.0 / D))
                    nc.vector.reciprocal(s1[:, 0:cw], s1[:, 0:cw])
                    fb = T.tile([128, CW2], F32, tag="fb")
                    nc.gpsimd.partition_broadcast(fb[:, 0:cw], s1[:, 0:cw])
                    for u in range(2):
                        y = ys[u]
                        nc.vector.tensor_mul(y[:, c0:c0 + cw],
                                             y[:, c0:c0 + cw].bitcast(F32),
                                             fb[:, 0:cw])
                        ta = T.tile([128, CW2], F32, tag="ropea")
                        tb = T.tile([128, CW2], F32, tag="ropeb")
                        tbs = T.tile([128, CW2], F32, tag="ropec")
                        yv = y[:, c0:c0 + cw].bitcast(F32)
                        nc.vector.tensor_mul(ta[:, 0:cw], yv, cos_sb[:, c0:c0 + cw])
                        nc.vector.tensor_mul(tb[:, 0:cw], yv, sin_sb[:, c0:c0 + cw])
                        nc.sync.dma_start(tbs[0:64, 0:cw], tb[64:128, 0:cw])
                        nc.sync.dma_start(tbs[64:128, 0:cw], tb[0:64, 0:cw])
                        nc.vector.tensor_sub(y[0:64, c0:c0 + cw],
                                             ta[0:64, 0:cw], tbs[0:64, 0:cw])
                        nc.vector.tensor_add(y[64:128, c0:c0 + cw],
                                             ta[64:128, 0:cw], tbs[64:128, 0:cw])

            # Wo arrives fp16 packed as [128,12,256]; upconvert into the
            # cos_sb slot (free after phase 2)
            wo16_sb = XP.tile([128, 2 * NKT, 128], F16, tag="xc", name="wo16_sb")
            nc.sync.dma_start(
                wo16_sb[:, 0:NKT, :], wall.ap()[3 * D:4 * D, :].rearrange(
                    "(p j) c -> p j c", p=128))
            nc.sync.dma_start(
                wo16_sb[:, NKT:2 * NKT, :], hfull[3 * D:4 * D, :].rearrange(
                    "(p j) c -> p j c", p=128))
            wo_sb = P.tile([128, 2 * NKT, 128], F32R, tag="cosslot", name="wo_sb")
            nc.vector.tensor_copy(wo_sb[:], wo16_sb[:])

            def wo_slice(u, m):
                return wo_sb[:, u * NKT + m, :]

            # ---------- phase 3: attention + partial o-projection ----------
            outp_loc = DR.tile([D, LP], F16)
            outr = outp_loc.rearrange("(mt p) l -> p mt l", p=128)
            for g in GROUPS:
                runts = []
                if g["runt"] is not None:
                    b = g["runt"]
                    a_lo = A0 + b * NAPB
                    s_row = S0 + b
                    for u in range(2):
                        kr = T.tile([128, 33], F32R, tag=f"kr{u}")
                        nc.vector.tensor_copy(kr[:, 0:32],
                                              y_k[u][:, a_lo:a_lo + 32].bitcast(F32))
                        nc.vector.tensor_copy(kr[:, 32:33],
                                              y_k[u][:, s_row:s_row + 1].bitcast(F32))
                        vr = T.tile([33, 256], F32R, tag=f"vr{u}")
                        # partition-shifting copies must go through DMA
                        nc.sync.dma_start(
                            vr[0:32, :], v_sb[32 * b:32 * b + 32, 28, :])
                        nc.sync.dma_start(
                            vr[32:33, :], v_sb[96 + b:97 + b, 28, :])
                        runts.append((kr, vr))

                kvts = g["kvt"] + ([None] if g["runt"] is not None else [])
                for (q0, qw) in g["q"]:
                    o_sb = []
                    for u in range(2):
                        oT_ps = PSY.tile([128, 512], F32, tag="vp", name="oT_ps")
                        sm_ps = PSY.tile([1, 512], F32, tag="ssqps", name="sm_ps")
                        for i, t in enumerate(kvts):
                            if t is None:
                                klhs = runts[u][0][:, :]
                                vlhs = runts[u][1][:, u * 128:(u + 1) * 128]
                                kvn = 33
                            else:
                                klhs = y_k[u][:, t * 128:(t + 1) * 128]
                                vlhs = v_sb[:, t, u * 128:(u + 1) * 128]
                                kvn = 128
                            s_ps = PSY.tile([128, 512], F32, tag="yp", name="s_ps")
                            nc.tensor.matmul(s_ps[0:kvn, 0:qw], klhs,
                                             y_q[u][:, q0:q0 + qw],
                                             start=True, stop=True)
                            pT = PT.tile([128, 512], F32R, tag="pT")
                            nc.scalar.activation(pT[0:kvn, 0:qw],
                                                 s_ps[0:kvn, 0:qw], AF.Exp,
                                                 scale=SCALE)
                            nc.tensor.matmul(oT_ps[:, 0:qw], vlhs, pT[0:kvn, 0:qw],
                                             start=(i == 0), stop=(i == len(kvts) - 1),
                                             skip_group_check=True)
                            nc.tensor.matmul(sm_ps[:, 0:qw], ones_r((slice(0, kvn), slice(0, 1))),
                                             pT[0:kvn, 0:qw],
                                             start=(i == 0), stop=(i == len(kvts) - 1),
                                             skip_group_check=True)
                        sm_sb = T.tile([1, 512], F32, tag="smsb")
                        nc.vector.reciprocal(sm_sb[:, 0:qw], sm_ps[:, 0:qw])
                        rb = T.tile([128, 512], F32, tag="rb")
                        nc.gpsimd.partition_broadcast(rb[:, 0:qw], sm_sb[:, 0:qw])
                        ot = OSB.tile([128, 512], F32R, tag="ot")
                        nc.vector.tensor_mul(ot[:, 0:qw], oT_ps[:, 0:qw], rb[:, 0:qw])
                        o_sb.append(ot)
                    for m in range(NKT):
                        op_ps = PSY.tile([128, 512], F32, tag="op", name="op_ps")
                        for u in range(2):
                            nc.tensor.matmul(
                                op_ps[:, 0:qw], wo_slice(u, m), o_sb[u][:, 0:qw],
                                start=(u == 0), stop=(u == 1))
                        op_sb = OSB.tile([128, 512], F16, tag="opsb")
                        nc.vector.tensor_copy(op_sb[:, 0:qw], op_ps[:, 0:qw])
                        nc.sync.dma_start(outr[:, m, q0:q0 + qw], op_sb[:, 0:qw])

            # state-token columns (3680:3683, plus discarded pad col
            # 3683: fp32r matmuls need width >= 4): o-projection of v_state^T
            for m in range(NKT):
                op_ps = PSY.tile([128, 512], F32, tag="op", name="op_ps_st")
                for u in range(2):
                    nc.tensor.matmul(op_ps[:, 0:4], wo_slice(u, m),
                                     o_state[u][:], start=(u == 0), stop=(u == 1))
                op_sb = OSB.tile([128, 512], F16, tag="opsb")
                nc.vector.tensor_copy(op_sb[:, 0:4], op_ps[:, 0:4])
                nc.sync.dma_start(outr[:, m, S0:S0 + 4], op_sb[:, 0:4])

            # zero the 3683:3712 pad columns so the ReduceScatter output is
            # garbage-free, then combine partial outputs on-device
            z16 = P.tile([128, 28], F16, tag="z16")
            nc.vector.memset(z16[:], 0.0)
            for m in range(NKT):
                nc.sync.dma_start(outr[:, m, L + 1:LP], z16[:])
            rs_out = DR.tile([ORD, LP], F16)
            nc.gpsimd.collective_compute(
                "ReduceScatter", mybir.AluOpType.add,
                replica_groups=[list(range(8))],
                ins=[outp_loc.opt()], outs=[rs_out.opt()])
            nc.sync.dma_start(outp.ap(), rs_out[:])

    nc.finalize()
    return nc


def _prep_inputs(x, freqs, freqs_action, freqs_state, Wq, bq, Wk, bk, Wv, bv,
                 Wo, bo, gq, gk):
    """Host-side input prep -> per-core in_maps. gq/gk are ones (per spec)."""
    x = np.asarray(x, np.float32)[0]
    xT16 = np.zeros((D, NS * SW), np.float16)
    xT16[:, :L] = x.T
    f = np.concatenate([np.asarray(freqs), np.asarray(freqs_action),
                        np.asarray(freqs_state)], 0).astype(np.float32)
    f = f.reshape(L, HD // 2, 2)
    cs16 = np.zeros((128, NS * SW), np.float16)
    cs16[0:64, :L] = f[..., 0].T
    cs16[64:128, :L] = f[..., 1].T
    perm = np.concatenate([np.arange(0, HD, 2), np.arange(1, HD, 2)])

    Wq = np.asarray(Wq, np.float32); Wk = np.asarray(Wk, np.float32)
    Wv = np.asarray(Wv, np.float32); Wo = np.asarray(Wo, np.float32)
    bq = np.asarray(bq, np.float32); bk = np.asarray(bk, np.float32)
    bv = np.asarray(bv, np.float32)

    in_maps = []
    for c in range(8):
        F, H = CORE_HEADS[c]
        pf = F * HD + perm
        ph = H * HD + perm
        vcols = np.r_[F * HD:(F + 1) * HD, H * HD:(H + 1) * HD]
        sl = slice(c * SW, (c + 1) * SW)
        brows = np.zeros((2, SW), np.float16)
        brows[0, 0:256] = bv[vcols]
        brows[1, :] = np.concatenate([bq[pf], bq[ph], bk[pf], bk[ph]])
        woF = Wo[F * HD:(F + 1) * HD, :].astype(np.float16).reshape(D, 128)
        hq = Wq[:, ph].astype(np.float16)
        hk = Wk[:, ph].astype(np.float16)
        hv = Wv[:, H * HD:(H + 1) * HD].astype(np.float16)
        hwo = (0.5 * Wo[H * HD:(H + 1) * HD, :]).astype(np.float16).reshape(D, 128)
        whalf = (np.concatenate([hq, hk], 0) if c % 2 == 0
                 else np.concatenate([hv, hwo], 0))
        in_maps.append({
            "gin": np.ascontiguousarray(np.concatenate(
                [xT16[:, sl], cs16[:, sl], brows], 0)),
            "wall": np.ascontiguousarray(np.concatenate(
                [Wq[:, pf].astype(np.float16),
                 Wk[:, pf].astype(np.float16),
                 Wv[:, F * HD:(F + 1) * HD].astype(np.float16),
                 woF], 0)),
            "whalf": np.ascontiguousarray(whalf),
        })
    return in_maps


def kernel(**inputs) -> np.ndarray:
    from concourse.bass_utils import run_bass_kernel_spmd

    if "nc" not in _PROGRAM_CACHE:
        _PROGRAM_CACHE["nc"] = _build_program()
    nc = _PROGRAM_CACHE["nc"]

    in_maps = _prep_inputs(**inputs)
    acc = None
    for attempt in range(3):
        try:
            res = run_bass_kernel_spmd(nc, in_maps, core_ids=list(range(8)))
            a = np.concatenate([res.results[c]["outp"] for c in range(8)],
                               0).astype(np.float32)
        except Exception:
            if attempt == 2:
                raise
            continue
        acc = a
        # transient device flakes can surface as non-finite values; relaunch
        if np.isfinite(a[:, :L]).all():
            break
    assert acc is not None

    bo = np.asarray(inputs["bo"], np.float32)
    out = acc[:, :L].T + bo[None, :]
    return out[None].astype(np.float32)


# revision 10
# speedup vs baseline: 7.9980x; 1.0063x over previous
"""CausalWanSelfAttention Trainium2 kernel — single SPMD launch on 8 NeuronCores.

Sharding: column-parallel QKV by heads. Each core owns 2 heads: one exclusive
"F" head plus one boundary "H" head shared with a sibling core; the H head's
output-projection weight is pre-scaled by 0.5 (and its RMSNorm sum-of-squares
contribution weighted 0.5) so summing the 8 partial outputs / statistics is
exact. RMSNorm statistics are combined with one tiny cross-core AllReduce
(2x3712 floats). The block-sparse mask decomposes into 4 dense attention
groups (no masking inside a group), so softmax runs without max-subtraction
(scores are O(1) after RMSNorm; |s| <= sqrt(128)). Scores are computed in
[kv, q] layout; softmax denominators via a ones-matmul; per-query
normalization is fused into the PSUM->SBUF copy. Head dims are permuted
(even dims then odd dims) host-side so RoPE needs no strided ops. State
tokens attend only to themselves (softmax==1 -> o=v): their three output
columns are produced on-chip from v_state^T = Wv^T x_state^T.

Host<->device traffic is minimized (the launch is transfer-bound over the
axon tunnel): x/cos/sin/biases ship fp16 SHARDED over tokens (1/8 per core,
packed into one tensor) and are AllGathered on-device; all four weight
matrices ship fp16 packed in one per-core tensor; the partial o-projection
outputs are combined on-device with an fp16 ReduceScatter so each core
returns only a 1/8 row-slice. Projection matmuls run fp16 (fp32 PSUM
accumulation); attention runs float32r.
"""
import os
import sys
import numpy as np

sys.path.insert(0, "/opt/trn_rl_repo")
# skip NEFF debug-info emission in the per-launch walrus compile (~70ms/launch)
os.environ.setdefault("CONCOURSE_SCRUB_NEFF_DEBUG_INFO", "1")

# ---- problem constants (hardcoded; kernel.py must be self-contained) ----
FS = 512
NIB = 3
NAPB = 32
L = 3683
LP = 3712           # 29 * 128
D = 1536
NH = 12
HD = 128
EPS = 1e-6
IB0 = FS                  # 512  image blocks start
A0 = FS + NIB * 2 * FS    # 3584 actions start
S0 = A0 + NIB * NAPB      # 3680 states start
NKT = D // 128            # 12 contraction tiles
NLT = LP // 128           # 29 L tiles
SCALE = float(1.0 / np.sqrt(HD))

SW = 512                  # per-core token shard width (8*512 = 4096, padded)
NS = 8
RW = [SW] * 7 + [LP - 7 * SW]   # real token width per shard (last: 128)
GROWS = D + 64 + 64 + 2   # shard rows: x(1536) + cos64 + sin64 + bv + bqk
ORD = D // 8              # 192 output rows per core after ReduceScatter

CW2 = 256  # rope/normalize L-chunk width


def _mk_chunks(w):
    ch = [(i * w, w) for i in range(LP // w)] + [(LP - LP % w, LP % w)]
    return [(c, x) for (c, x) in ch if x > 0]

CHUNKS2 = _mk_chunks(CW2)

# core -> (F head, H head); H heads are computed on two cores each
CORE_HEADS = []
for _a in range(4):
    CORE_HEADS.append((3 * _a, 3 * _a + 1))
    CORE_HEADS.append((3 * _a + 2, 3 * _a + 1))


def _groups():
    """Dense attention groups: q ranges, kv 128-tile indices, runt kv info."""
    gs = [dict(q=[(0, 512)], kvt=list(range(4)), runt=None)]
    for b in range(NIB):
        be = IB0 + (b + 1) * 2 * FS
        kv0 = max(IB0, be - 4 * FS)
        if kv0 == IB0:
            tiles = list(range(be // 128))
        else:
            tiles = list(range(4)) + list(range(kv0 // 128, be // 128))
        q = [(IB0 + b * 2 * FS, 512), (IB0 + b * 2 * FS + 512, 512),
             (A0 + b * NAPB, NAPB)]
        gs.append(dict(q=q, kvt=tiles, runt=b))
    return gs

GROUPS = _groups()

_PROGRAM_CACHE = {}


def _build_program():
    import concourse.bacc as bacc
    import concourse.tile as tile
    from concourse import mybir

    F16 = mybir.dt.float16
    F32 = mybir.dt.float32
    F32R = mybir.dt.float32r
    AF = mybir.ActivationFunctionType

    nc = bacc.Bacc("TRN2", target_bir_lowering=False, debug=False, num_devices=8)

    gin = nc.dram_tensor("gin", [GROWS, SW], F16, kind="ExternalInput")
    wall = nc.dram_tensor("wall", [4 * D, 128], F16, kind="ExternalInput")
    whalf = nc.dram_tensor("whalf", [2 * D, 128], F16, kind="ExternalInput")

    outp = nc.dram_tensor("outp", [ORD, L], F16, kind="ExternalOutput")

    with tile.TileContext(nc) as tc:
        with tc.tile_pool(name="persist", bufs=1) as P, \
             tc.tile_pool(name="xin", bufs=2) as XP, \
             tc.tile_pool(name="tmp", bufs=2) as T, \
             tc.tile_pool(name="pt", bufs=3) as PT, \
             tc.tile_pool(name="osb", bufs=2) as OSB, \
             tc.tile_pool(name="ps", bufs=2, space="PSUM") as PSY, \
             tc.tile_pool(name="dram", bufs=1, space="DRAM") as DR:

            # ---------- phase-1-resident SBUF ----------
            wq_sb = P.tile([128, NKT, 256], F16, tag="wq")
            wk_sb = P.tile([128, NKT, 256], F16, tag="wk")
            wv_sb = P.tile([128, NKT, 256], F16, tag="wv")
            bqk_sb = P.tile([128, 4], F32, tag="bqk")
            bv_sb = P.tile([128, 256], F32, tag="bv")
            bvT = P.tile([128, 2], F32, tag="bvT")
            ones2 = P.tile([128, 2], F32, tag="ones2")
            # whole-kernel-resident
            y_q = [P.tile([128, LP], F32R, tag=f"yq{u}", name=f"yq{u}") for u in range(2)]
            y_k = [P.tile([128, LP], F32R, tag=f"yk{u}", name=f"yk{u}") for u in range(2)]
            v_sb = P.tile([128, NLT, 256], F32R, tag="vsb")
            cos_sb = P.tile([128, LP], F32, tag="cosslot", name="cos_sb")
            sin_sb = P.tile([128, LP], F32, tag="sinslot", name="sin_sb")

            for t, w_sb in enumerate((wq_sb, wk_sb, wv_sb)):
                nc.sync.dma_start(
                    w_sb[:, :, 0:128], wall.ap()[t * D:(t + 1) * D, :].rearrange(
                        "(kt p) c -> p kt c", p=128))
            # biases arrive fp16 inside this core's own (pre-gather) shard
            bqk16 = T.tile([128, 4], F16, tag="bqk16")
            nc.sync.dma_start(
                bqk16[:], gin.ap()[D + 129, :].rearrange("(i p) -> p i", p=128))
            nc.vector.tensor_copy(bqk_sb[:], bqk16[:])
            bv16 = T.tile([1, 256], F16, tag="bv16")
            nc.sync.dma_start(bv16[:], gin.ap()[D + 128:D + 129, 0:256])
            bv1f = T.tile([1, 256], F32, tag="bv1f")
            nc.vector.tensor_copy(bv1f[:], bv16[:])
            nc.gpsimd.partition_broadcast(bv_sb[:], bv1f[:])
            bvT16 = T.tile([128, 2], F16, tag="bvT16")
            nc.sync.dma_start(
                bvT16[:], gin.ap()[D + 128, 0:256].rearrange("(u p) -> p u", p=128))
            nc.vector.tensor_copy(bvT[:], bvT16[:])
            nc.vector.memset(ones2[:, 0:1], 1.0)
            nc.vector.memset(ones2[:, 1:2], 0.5)

            def ones_r(sl):
                return ones2[sl].bitcast(F32R)

            # ---------- collective: AllGather x/cos/sin shards ----------
            # (collectives cannot touch IO tensors directly -> stage via
            # internal DRAM)
            gstage = DR.tile([GROWS, SW], F16)
            nc.sync.dma_start(gstage[:], gin.ap())
            ging = DR.tile([NS * GROWS, SW], F16, addr_space="Shared")
            nc.gpsimd.collective_compute(
                "AllGather", mybir.AluOpType.bypass,
                replica_groups=[list(range(8))],
                ins=[gstage.opt()], outs=[ging])

            # shared H-head weights: each pair member ships half, pair-wise
            # AllGather reconstructs [hq; hk; hv; hwo] identically on both
            whstage = DR.tile([2 * D, 128], F16)
            nc.sync.dma_start(whstage[:], whalf.ap())
            hfull = DR.tile([4 * D, 128], F16)
            nc.gpsimd.collective_compute(
                "AllGather", mybir.AluOpType.bypass,
                replica_groups=[[2 * a, 2 * a + 1] for a in range(4)],
                ins=[whstage.opt()], outs=[hfull])
            for t, w_sb in enumerate((wq_sb, wk_sb, wv_sb)):
                nc.sync.dma_start(
                    w_sb[:, :, 128:256], hfull[t * D:(t + 1) * D, :].rearrange(
                        "(kt p) c -> p kt c", p=128))

            # cos/sin: fp16 gathered [64, rw] per shard -> duplicated halves,
            # converted to fp32
            for s in range(NS):
                c0, rw = s * SW, RW[s]
                for j, dst in enumerate((cos_sb, sin_sb)):
                    src = ging[s * GROWS + D + 64 * j:s * GROWS + D + 64 * (j + 1), :]
                    cst = T.tile([128, SW], F16, tag="cst")
                    nc.sync.dma_start(cst[0:64, 0:rw], src[:, 0:rw])
                    nc.sync.dma_start(cst[64:128, 0:rw], src[:, 0:rw])
                    nc.vector.tensor_copy(dst[:, c0:c0 + rw], cst[:, 0:rw])

            # ---------- phase 1: projections + ssq partials ----------
            cin = DR.tile([1, 2 * LP], F32)
            cout = DR.tile([1, 2 * LP], F32)
            for s in range(NS):
                c0, rw = s * SW, RW[s]
                xpart = ging[s * GROWS:s * GROWS + D, :].rearrange(
                    "(kt p) l -> p kt l", p=128)
                xc = XP.tile([128, NKT, SW], F16, tag="xc")
                nc.sync.dma_start(xc[:, :, 0:rw], xpart[:, :, 0:rw])
                for ti, (w_sb, ys) in enumerate([(wq_sb, y_q), (wk_sb, y_k)]):
                    ssq_ps = PSY.tile([1, 512], F32, tag="ssqps")
                    for u in range(2):
                        yp = PSY.tile([128, 512], F32, tag="yp")
                        for kt in range(NKT):
                            nc.tensor.matmul(
                                yp[:, 0:rw], w_sb[:, kt, u * 128:(u + 1) * 128],
                                xc[:, kt, 0:rw],
                                start=(kt == 0), stop=(kt == NKT - 1))
                        nc.vector.tensor_scalar_add(
                            ys[u][:, c0:c0 + rw], yp[:, 0:rw],
                            bqk_sb[:, 2 * ti + u:2 * ti + u + 1])
                        y2 = T.tile([128, SW], F32R, tag="y2")
                        nc.scalar.activation(y2[:, 0:rw],
                                             ys[u][:, c0:c0 + rw].bitcast(F32),
                                             AF.Square)
                        nc.tensor.matmul(ssq_ps[:, 0:rw], ones_r((slice(None), slice(u, u + 1))),
                                         y2[:, 0:rw], start=(u == 0), stop=(u == 1),
                                         skip_group_check=True)
                    ssq_st = T.tile([1, SW], F32, tag="ssqst")
                    nc.vector.tensor_copy(ssq_st[:, 0:rw], ssq_ps[:, 0:rw])
                    nc.sync.dma_start(cin[0:1, ti * LP + c0:ti * LP + c0 + rw], ssq_st[:, 0:rw])
                for lt in range(c0 // 128, (c0 + rw) // 128):
                    vp = PSY.tile([128, 512], F32, tag="vp", name="vp")[:, 0:256]
                    loff = lt * 128 - c0
                    for kt in range(NKT):
                        nc.tensor.matmul(vp[:], xc[:, kt, loff:loff + 128],
                                         wv_sb[:, kt, :],
                                         start=(kt == 0), stop=(kt == NKT - 1))
                    nc.vector.tensor_add(v_sb[:, lt, :], vp[:], bv_sb[:])

            # state tokens (3680:3683, in shard 7 cols 96:99): o = v, computed
            # transposed as v^T = Wv^T x^T so it feeds the o-projection directly
            xst = T.tile([128, NKT, 4], F16, tag="xst")
            nc.sync.dma_start(
                xst[:], ging[7 * GROWS:7 * GROWS + D, :].rearrange(
                    "(kt p) l -> p kt l", p=128)[:, :, 96:100])
            o_state = [P.tile([128, 4], F32R, tag=f"ost{u}", name=f"ost{u}")
                       for u in range(2)]
            for u in range(2):
                vs_ps = PSY.tile([128, 512], F32, tag="yp", name="vs_ps")
                for kt in range(NKT):
                    nc.tensor.matmul(vs_ps[:, 0:4],
                                     wv_sb[:, kt, u * 128:(u + 1) * 128],
                                     xst[:, kt, :],
                                     start=(kt == 0), stop=(kt == NKT - 1))
                nc.vector.tensor_scalar_add(o_state[u][:], vs_ps[:, 0:4],
                                            bvT[:, u:u + 1])

            # ---------- collective: AllReduce the ssq partials ----------
            nc.gpsimd.collective_compute(
                "AllReduce", mybir.AluOpType.add,
                replica_groups=[list(range(8))],
                ins=[cin.opt()], outs=[cout.opt()])
            eps_t = P.tile([1, 1], F32, tag="epst")
            nc.vector.memset(eps_t[:], float(EPS))

            # ---------- phase 2: normalize + rope (in place on y) ----------
            for (c0, cw) in CHUNKS2:
                for ti, ys in enumerate([y_q, y_k]):
                    s1 = T.tile([1, CW2], F32, tag="s1")
                    nc.sync.dma_start(s1[:, 0:cw],
                                      cout[0:1, ti * LP + c0:ti * LP + c0 + cw])
                    nc.scalar.activation(s1[:, 0:cw], s1[:, 0:cw], AF.Sqrt,
                                         bias=eps_t[:, 0:1], scale=float(1.0 / D))
                    nc.vector.reciprocal(s1[:, 0:cw], s1[:, 0:cw])
                    fb = T.tile([128, CW2], F32, tag="fb")
                    nc.gpsimd.partition_broadcast(fb[:, 0:cw], s1[:, 0:cw])
                    for u in range(2):
                        y = ys[u]
                        nc.vector.tensor_mul(y[:, c0:c0 + cw],
                                             y[:, c0:c0 + cw].bitcast(F32),
                                             fb[:, 0:cw])
                        ta = T.tile([128, CW2], F32, tag="ropea")
                        tb = T.tile([128, CW2], F32, tag="ropeb")
                        tbs = T.tile([128, CW2], F32, tag="ropec")
                        yv = y[:, c0:c0 + cw].bitcast(F32)
                        nc.vector.tensor_mul(ta[:, 0:cw], yv, cos_sb[:, c0:c0 + cw])
                        nc.vector.tensor_mul(tb[:, 0:cw], yv, sin_sb[:, c0:c0 + cw])
                        nc.sync.dma_start(tbs[0:64, 0:cw], tb[64:128, 0:cw])
                        nc.sync.dma_start(tbs[64:128, 0:cw], tb[0:64, 0:cw])
                        nc.vector.tensor_sub(y[0:64, c0:c0 + cw],
                                             ta[0:64, 0:cw], tbs[0:64, 0:cw])
                        nc.vector.tensor_add(y[64:128, c0:c0 + cw],
                                             ta[64:128, 0:cw], tbs[64:128, 0:cw])

            # Wo arrives fp16 packed as [128,12,256]; upconvert into the
            # cos_sb slot (free after phase 2)
            wo16_sb = XP.tile([128, 2 * NKT, 128], F16, tag="xc", name="wo16_sb")
            nc.sync.dma_start(
                wo16_sb[:, 0:NKT, :], wall.ap()[3 * D:4 * D, :].rearrange(
                    "(p j) c -> p j c", p=128))
            nc.sync.dma_start(
                wo16_sb[:, NKT:2 * NKT, :], hfull[3 * D:4 * D, :].rearrange(
                    "(p j) c -> p j c", p=128))
            wo_sb = P.tile([128, 2 * NKT, 128], F32R, tag="cosslot", name="wo_sb")
            nc.vector.tensor_copy(wo_sb[:], wo16_sb[:])

            def wo_slice(u, m):
                return wo_sb[:, u * NKT + m, :]

            # ---------- phase 3: attention + partial o-projection ----------
            outp_loc = DR.tile([D, LP], F16)
            outr = outp_loc.rearrange("(mt p) l -> p mt l", p=128)
            for g in GROUPS:
                runts = []
                if g["runt"] is not None:
                    b = g["runt"]
                    a_lo = A0 + b * NAPB
                    s_row = S0 + b
                    for u in range(2):
                        kr = T.tile([128, 33], F32R, tag=f"kr{u}")
                        nc.vector.tensor_copy(kr[:, 0:32],
                                              y_k[u][:, a_lo:a_lo + 32].bitcast(F32))
                        nc.vector.tensor_copy(kr[:, 32:33],
                                              y_k[u][:, s_row:s_row + 1].bitcast(F32))
                        vr = T.tile([33, 256], F32R, tag=f"vr{u}")
                        # partition-shifting copies must go through DMA
                        nc.sync.dma_start(
                            vr[0:32, :], v_sb[32 * b:32 * b + 32, 28, :])
                        nc.sync.dma_start(
                            vr[32:33, :], v_sb[96 + b:97 + b, 28, :])
                        runts.append((kr, vr))

                kvts = g["kvt"] + ([None] if g["runt"] is not None else [])
                for (q0, qw) in g["q"]:
                    o_sb = []
                    for u in range(2):
                        oT_ps = PSY.tile([128, 512], F32, tag="vp", name="oT_ps")
                        sm_ps = PSY.tile([1, 512], F32, tag="ssqps", name="sm_ps")
                        for i, t in enumerate(kvts):
                            if t is None:
                                klhs = runts[u][0][:, :]
                                vlhs = runts[u][1][:, u * 128:(u + 1) * 128]
                                kvn = 33
                            else:
                                klhs = y_k[u][:, t * 128:(t + 1) * 128]
                                vlhs = v_sb[:, t, u * 128:(u + 1) * 128]
                                kvn = 128
                            s_ps = PSY.tile([128, 512], F32, tag="yp", name="s_ps")
                            nc.tensor.matmul(s_ps[0:kvn, 0:qw], klhs,
                                             y_q[u][:, q0:q0 + qw],
                                             start=True, stop=True)
                            pT = PT.tile([128, 512], F32R, tag="pT")
                            nc.scalar.activation(pT[0:kvn, 0:qw],
                                                 s_ps[0:kvn, 0:qw], AF.Exp,
                                                 scale=SCALE)
                            nc.tensor.matmul(oT_ps[:, 0:qw], vlhs, pT[0:kvn, 0:qw],
                                             start=(i == 0), stop=(i == len(kvts) - 1),
                                             skip_group_check=True)
                            nc.tensor.matmul(sm_ps[:, 0:qw], ones_r((slice(0, kvn), slice(0, 1))),
                                             pT[0:kvn, 0:qw],
                                             start=(i == 0), stop=(i == len(kvts) - 1),
                                             skip_group_check=True)
                        sm_sb = T.tile([1, 512], F32, tag="smsb")
                        nc.vector.reciprocal(sm_sb[:, 0:qw], sm_ps[:, 0:qw])
                        rb = T.tile([128, 512], F32, tag="rb")
                        nc.gpsimd.partition_broadcast(rb[:, 0:qw], sm_sb[:, 0:qw])
                        ot = OSB.tile([128, 512], F32R, tag="ot")
                        nc.vector.tensor_mul(ot[:, 0:qw], oT_ps[:, 0:qw], rb[:, 0:qw])
                        o_sb.append(ot)
                    for m in range(NKT):
                        op_ps = PSY.tile([128, 512], F32, tag="op", name="op_ps")
                        for u in range(2):
                            nc.tensor.matmul(
                                op_ps[:, 0:qw], wo_slice(u, m), o_sb[u][:, 0:qw],
                                start=(u == 0), stop=(u == 1))
                        op_sb = OSB.tile([128, 512], F16, tag="opsb")
                        nc.vector.tensor_copy(op_sb[:, 0:qw], op_ps[:, 0:qw])
                        nc.sync.dma_start(outr[:, m, q0:q0 + qw], op_sb[:, 0:qw])

            # state-token columns (3680:3683, plus discarded pad col
            # 3683: fp32r matmuls need width >= 4): o-projection of v_state^T
            for m in range(NKT):
                op_ps = PSY.tile([128, 512], F32, tag="op", name="op_ps_st")
                for u in range(2):
                    nc.tensor.matmul(op_ps[:, 0:4], wo_slice(u, m),
                                     o_state[u][:], start=(u == 0), stop=(u == 1))
                op_sb = OSB.tile([128, 512], F16, tag="opsb")
                nc.vector.tensor_copy(op_sb[:, 0:4], op_ps[:, 0:4])
                nc.sync.dma_start(outr[:, m, S0:S0 + 4], op_sb[:, 0:4])

            # zero the 3683:3712 pad columns so the ReduceScatter output is
            # garbage-free, then combine partial outputs on-device
            z16 = P.tile([128, 28], F16, tag="z16")
            nc.vector.memset(z16[:], 0.0)
            for m in range(NKT):
                nc.sync.dma_start(outr[:, m, L + 1:LP], z16[:])
            rs_out = DR.tile([ORD, LP], F16)
            nc.gpsimd.collective_compute(
                "ReduceScatter", mybir.AluOpType.add,
                replica_groups=[list(range(8))],
                ins=[outp_loc.opt()], outs=[rs_out.opt()])
            nc.sync.dma_start(outp.ap(), rs_out[:, 0:L])

    nc.finalize()
    return nc


def _prep_inputs(x, freqs, freqs_action, freqs_state, Wq, bq, Wk, bk, Wv, bv,
                 Wo, bo, gq, gk):
    """Host-side input prep -> per-core in_maps. gq/gk are ones (per spec)."""
    x = np.asarray(x, np.float32)[0]
    xT16 = np.zeros((D, NS * SW), np.float16)
    xT16[:, :L] = x.T
    f = np.concatenate([np.asarray(freqs), np.asarray(freqs_action),
                        np.asarray(freqs_state)], 0).astype(np.float32)
    f = f.reshape(L, HD // 2, 2)
    cs16 = np.zeros((128, NS * SW), np.float16)
    cs16[0:64, :L] = f[..., 0].T
    cs16[64:128, :L] = f[..., 1].T
    perm = np.concatenate([np.arange(0, HD, 2), np.arange(1, HD, 2)])

    Wq = np.asarray(Wq, np.float32); Wk = np.asarray(Wk, np.float32)
    Wv = np.asarray(Wv, np.float32); Wo = np.asarray(Wo, np.float32)
    bq = np.asarray(bq, np.float32); bk = np.asarray(bk, np.float32)
    bv = np.asarray(bv, np.float32)

    in_maps = []
    for c in range(8):
        F, H = CORE_HEADS[c]
        pf = F * HD + perm
        ph = H * HD + perm
        vcols = np.r_[F * HD:(F + 1) * HD, H * HD:(H + 1) * HD]
        sl = slice(c * SW, (c + 1) * SW)
        brows = np.zeros((2, SW), np.float16)
        brows[0, 0:256] = bv[vcols]
        brows[1, :] = np.concatenate([bq[pf], bq[ph], bk[pf], bk[ph]])
        woF = Wo[F * HD:(F + 1) * HD, :].astype(np.float16).reshape(D, 128)
        hq = Wq[:, ph].astype(np.float16)
        hk = Wk[:, ph].astype(np.float16)
        hv = Wv[:, H * HD:(H + 1) * HD].astype(np.float16)
        hwo = (0.5 * Wo[H * HD:(H + 1) * HD, :]).astype(np.float16).reshape(D, 128)
        whalf = (np.concatenate([hq, hk], 0) if c % 2 == 0
                 else np.concatenate([hv, hwo], 0))
        in_maps.append({
            "gin": np.ascontiguousarray(np.concatenate(
                [xT16[:, sl], cs16[:, sl], brows], 0)),
            "wall": np.ascontiguousarray(np.concatenate(
                [Wq[:, pf].astype(np.float16),
                 Wk[:, pf].astype(np.float16),
                 Wv[:, F * HD:(F + 1) * HD].astype(np.float16),
                 woF], 0)),
            "whalf": np.ascontiguousarray(whalf),
        })
    return in_maps


def kernel(**inputs) -> np.ndarray:
    from concourse.bass_utils import run_bass_kernel_spmd

    if "nc" not in _PROGRAM_CACHE:
        _PROGRAM_CACHE["nc"] = _build_program()
    nc = _PROGRAM_CACHE["nc"]

    in_maps = _prep_inputs(**inputs)
    acc = None
    for attempt in range(3):
        try:
            res = run_bass_kernel_spmd(nc, in_maps, core_ids=list(range(8)))
            a = np.concatenate([res.results[c]["outp"] for c in range(8)],
                               0).astype(np.float32)
        except Exception:
            if attempt == 2:
                raise
            continue
        acc = a
        # transient device flakes can surface as non-finite values; relaunch
        if np.isfinite(a).all():
            break
    assert acc is not None

    bo = np.asarray(inputs["bo"], np.float32)
    out = acc.T + bo[None, :]
    return out[None].astype(np.float32)
